# revision 35
# baseline (speedup 1.0000x reference)
"""Trainium2 Bass kernel for a 4-block transformer decoder (nn_Decoder).

Strategy: data-parallel over batch across 8 NeuronCores (1 batch element per
core), no collectives. Per core the whole decoder runs on [T=256, D=512]
activations kept feature-major ("TN layout": features on SBUF partitions,
tokens on the free dim), so every linear layer consumes weights as the
stationary matmul operand directly in their natural [in, out] layout.
Weights are converted to bf16 on the host (halves HBM traffic; fp32 matmul
on TRN2 runs at 1/4 rate); accumulation stays fp32 in PSUM and the residual
stream / softmax / layernorm statistics stay fp32.
"""

import numpy as np
import ml_dtypes

import bass_rust
import concourse.bass as bass
import concourse.mybir as mybir
from concourse.tile import TileContext
from concourse.masks import make_identity
from concourse.bass_utils import run_bass_kernel_spmd

DT = mybir.dt
BF = DT.bfloat16
F32 = DT.float32
AF = mybir.ActivationFunctionType
AX = mybir.AxisListType
OP = mybir.AluOpType
BF_NP = ml_dtypes.bfloat16

# Model dims (fixed by the problem)
V = 32000
D = 512
H = 8
NB = 4
B = 8
S = 256
T = 256
DK = D // 128          # 4 k-tiles over the model dim
TK = T // 128          # 2 token tiles
HD = H * D             # 4096 concat-head dim
FF = 4 * D             # 2048
VOC = V + 1            # 32001
VCH = 512              # vocab free-dim chunk
NEG = -1.0e30
RSQ = 1.0 / float(np.sqrt(np.float32(D)))

# bias-column layout inside bias_cols[:, block, col]
_BC_Q_S, _BC_K_S, _BC_V_S, _BC_O_S = 0, 32, 64, 96
_BC_Q_C, _BC_K_C, _BC_V_C, _BC_O_C = 100, 132, 164, 196
_BC_B1, _BC_B2 = 200, 216
_BC_W = 220


def _split_excess_waits(nc, max_waits=1):
    """walrus in this container encodes at most one semaphore wait per
    instruction; move extra waits onto same-engine carrier nops."""
    for bb in nc.main_func.blocks:
        insts = bb.instructions
        def nwaits(ins):
            si = ins.sync_info
            return len(si.on_wait) if si is not None else 0
        if not any(nwaits(i) > max_waits for i in insts):
            continue
        new_list = []
        for ins in list(insts):
            si0 = ins.sync_info
            waits = list(si0.on_wait) if si0 is not None else []
            if len(waits) > max_waits:
                excess = waits[: len(waits) - max_waits]
                keep = waits[len(waits) - max_waits:]
                eng = nc.engines[ins.engine]
                for i in range(0, len(excess), max_waits):
                    chunk = excess[i:i + max_waits]
                    carrier = eng.nop(nofuse=True, hint="wait_split")
                    cins = carrier.ins
                    cur = nc.cur_bb.bb.instructions
                    assert cur[-1].name == cins.name
                    cur.pop()
                    cins.sync_info = bass_rust.SyncInfo(on_wait=chunk, on_update=[])
                    new_list.append(cins)
                si0.on_wait = keep
            new_list.append(ins)
        insts[:] = new_list


def build_decoder_nc(flags):
    """flags: dict with booleans use_mask2, bias_* (see _compute_flags)."""
    nc = bass.Bass()

    x0t = nc.declare_dram_parameter("x0t", [D, T], F32, isOutput=False)
    encst = nc.declare_dram_parameter("encst", [D, S], BF, isOutput=False)
    mask1 = nc.declare_dram_parameter("mask1", [T, T], F32, isOutput=False)
    if flags["use_mask2"]:
        mask2 = nc.declare_dram_parameter("mask2", [T, S], F32, isOutput=False)
    bias_cols = nc.declare_dram_parameter("bias_cols", [128, NB, _BC_W], F32, isOutput=False)
    lngb = nc.declare_dram_parameter("lngb", [128, NB, 3, 2, DK], F32, isOutput=False)
    wqkv_s = [nc.declare_dram_parameter(f"wqkv_s{b}", [3, H, D, D], BF, isOutput=False) for b in range(NB)]
    wqkv_c = [nc.declare_dram_parameter(f"wqkv_c{b}", [3, H, D, D], BF, isOutput=False) for b in range(NB)]
    wo_s = [nc.declare_dram_parameter(f"wo_s{b}", [HD, D], BF, isOutput=False) for b in range(NB)]
    wo_c = [nc.declare_dram_parameter(f"wo_c{b}", [HD, D], BF, isOutput=False) for b in range(NB)]
    w1 = [nc.declare_dram_parameter(f"w1_{b}", [D, FF], BF, isOutput=False) for b in range(NB)]
    w2 = [nc.declare_dram_parameter(f"w2_{b}", [FF, D], BF, isOutput=False) for b in range(NB)]
    wout = nc.declare_dram_parameter("wout", [D, VOC], BF, isOutput=False)

    y = nc.declare_dram_parameter("y", [T, VOC], DT.float16, isOutput=True)
    attw = nc.declare_dram_parameter("attw", [H * T, S], F32, isOutput=True)

    with TileContext(nc) as tc:
        with (
            tc.tile_pool(name="const", bufs=1) as const,
            tc.tile_pool(name="persist", bufs=1) as persist,
            tc.tile_pool(name="wpool", bufs=2) as wpool,
            tc.tile_pool(name="apool", bufs=2) as apool,
            tc.tile_pool(name="sm", bufs=3) as sm,
            tc.tile_pool(name="pmid", bufs=3, space="PSUM") as pmid,
            tc.tile_pool(name="pbig", bufs=2, space="PSUM") as pbig,
            tc.tile_pool(name="ptr", bufs=2, space="PSUM") as ptr,
            tc.tile_pool(name="pst", bufs=1, space="PSUM") as pst,
        ):
            # ---- constants ----
            ident = const.tile([128, 128], BF)
            make_identity(nc, ident)
            ones_col = const.tile([128, 1], BF)
            nc.vector.memset(ones_col, 1.0)
            ones_row_f = const.tile([1, 128], F32)
            nc.vector.memset(ones_row_f, 1.0)
            ones_row_b = const.tile([1, 128], BF)
            nc.vector.memset(ones_row_b, 1.0)
            eps1 = const.tile([1, 1], F32)
            nc.vector.memset(eps1, 1e-5)

            # ---- persistent state ----
            res = persist.tile([128, DK, T], F32)       # residual stream (xT)
            xnbf = persist.tile([128, DK, T], BF)       # current matmul input (xT, bf16)
            encst_sb = persist.tile([128, DK, S], BF)
            obf = persist.tile([128, H * DK, T], BF)    # concat head outputs (oT)
            h1t = persist.tile([128, FF // 128, T], BF)
            tbf = persist.tile([128, DK, T], BF)        # LN scratch: bf16 copy of res
            sqbf = persist.tile([128, DK, T], BF)       # LN scratch: squares
            mask1_sb = persist.tile([128, TK, T], F32)
            if flags["use_mask2"]:
                mask2_sb = persist.tile([128, TK, S], F32)
            bias_sb = persist.tile([128, NB, _BC_W], F32)
            lngb_sb = persist.tile([128, NB, 3, 2, DK], F32)

            # ---- input DMA ----
            nc.sync.dma_start(out=res, in_=x0t.rearrange("(kt p) t -> p kt t", p=128))
            nc.sync.dma_start(out=encst_sb, in_=encst.rearrange("(kt p) t -> p kt t", p=128))
            nc.sync.dma_start(out=mask1_sb, in_=mask1.rearrange("(qt p) k -> p qt k", p=128))
            if flags["use_mask2"]:
                nc.sync.dma_start(out=mask2_sb, in_=mask2.rearrange("(qt p) k -> p qt k", p=128))
            nc.sync.dma_start(out=bias_sb, in_=bias_cols[:, :, :])
            nc.sync.dma_start(out=lngb_sb, in_=lngb[:, :, :, :, :])

            for c in range(DK):
                nc.gpsimd.tensor_copy(xnbf[:, c, :], res[:, c, :])

            def bias_col(b, base, idx):
                return bias_sb[:, b, base + idx:base + idx + 1]

            def ln_col(b, i, gb, c):
                return lngb_sb[:, b, i, gb, c:c + 1]

            # ---------- building blocks ----------

            def proj_tn(wtile, x_in, out_tile, out_idx, b, bias_base, h,
                        scale, use_bias, engine_alt=False):
                """TN projection: out[:, out_idx+n, :] = W.T @ x (+bias) for n in 0..DK-1."""
                for n in range(DK):
                    ps = pmid.tile([128, T], F32, tag="pmid", name="ps_proj")
                    for kt in range(DK):
                        nc.tensor.matmul(ps, wtile[:, kt, n * 128:(n + 1) * 128],
                                         x_in[:, kt, :], start=(kt == 0), stop=(kt == DK - 1))
                    dst = out_tile[:, out_idx + n, :]
                    if use_bias:
                        bcol = bias_col(b, bias_base, (h * DK + n) if h is not None else n)
                        nc.scalar.activation(out=dst, in_=ps, func=AF.Identity,
                                             bias=bcol, scale=scale)
                    elif scale != 1.0:
                        nc.scalar.mul(dst, ps, scale)
                    elif engine_alt:
                        nc.vector.tensor_copy(dst, ps)
                    else:
                        nc.scalar.copy(dst, ps)

            def v_proj(wtile, x_in, vout):
                """natural-layout value projection: v[t, e] (bias folded in at oT)."""
                for tk in range(TK):
                    ps = pbig.tile([128, D], F32, tag="pbig", name="ps_v")
                    for kt in range(DK):
                        nc.tensor.matmul(ps, x_in[:, kt, tk * 128:(tk + 1) * 128],
                                         wtile[:, kt, :], start=(kt == 0), stop=(kt == DK - 1))
                    nc.vector.tensor_copy(vout[:, tk, :], ps)

            def attention(b, is_self, emit_att, pstat, pre=None):
                """one MHA sublayer; reads xnbf (queries) and for cross the
                encoder states; accumulates output into res via Wo.

                Head loop is software-pipelined: head h+1's K/V/Q projections
                are emitted between head h's softmax and its transpose/oT so
                the PE stays busy during the softmax latency. K/V are
                projected before Q so that for cross-attention the (LN-
                independent) K/V matmuls can run while the preceding
                layernorm finishes."""
                wqkv = wqkv_s[b] if is_self else wqkv_c[b]
                wo = wo_s[b] if is_self else wo_c[b]
                kvx = xnbf if is_self else encst_sb
                klen = T if is_self else S
                kk = klen // 128
                q_base = _BC_Q_S if is_self else _BC_Q_C
                k_base = _BC_K_S if is_self else _BC_K_C
                v_base = _BC_V_S if is_self else _BC_V_C
                o_base = _BC_O_S if is_self else _BC_O_C
                ub_q = flags["bias_q_s" if is_self else "bias_q_c"][b]
                ub_k = flags["bias_k_s" if is_self else "bias_k_c"][b]
                ub_v = flags["bias_v_s" if is_self else "bias_v_c"][b]
                ub_o = flags["bias_o_s" if is_self else "bias_o_c"][b]
                msk = None
                if is_self:
                    msk = mask1_sb
                elif flags["use_mask2"]:
                    msk = mask2_sb

                def load_head(h):
                    # split K/V (used first) from Q so their prefetch depths
                    # decouple
                    wkv = wpool.tile([128, 2, DK, D], BF, tag="wkv", bufs=4, name="wkv")
                    for j, m in enumerate((1, 2)):
                        eng = nc.sync if (h + j) % 2 == 0 else nc.scalar
                        eng.dma_start(out=wkv[:, m - 1], in_=wqkv[m, h].rearrange(
                            "(kt p) n -> p kt n", p=128))
                    wq = wpool.tile([128, DK, D], BF, tag="wq", bufs=3, name="wq")
                    eng = nc.sync if h % 2 == 0 else nc.scalar
                    eng.dma_start(out=wq, in_=wqkv[0, h].rearrange(
                        "(kt p) n -> p kt n", p=128))
                    return wkv, wq

                def proj_kv(h, wkv):
                    kbf = apool.tile([128, DK, S], BF, tag="kbf", bufs=2, name="kbf")
                    vbf = apool.tile([128, kk, D], BF, tag="vbf", bufs=3, name="vbf")
                    proj_tn(wkv[:, 0], kvx, kbf, 0, b, k_base, h, 1.0, ub_k, engine_alt=True)
                    v_proj(wkv[:, 1], kvx, vbf)
                    return kbf, vbf

                def proj_head(h, wt, kv=None):
                    wkv, wq = wt
                    kbf, vbf = kv if kv is not None else proj_kv(h, wkv)
                    qbf = apool.tile([128, DK, T], BF, tag="qbf", bufs=2, name="qbf")
                    proj_tn(wq, xnbf, qbf, 0, b, q_base, h, RSQ, ub_q)
                    return qbf, kbf, vbf

                if pre is not None:
                    wt, kv0 = pre
                    nxt = proj_head(0, wt, kv0)
                else:
                    wt = load_head(0)
                    nxt = proj_head(0, wt)
                wt_next = load_head(1)
                for h in range(H):
                    qbf, kbf, vbf = nxt
                    # pure causal self-attention: queries in tile 0 only attend
                    # keys in tile 0, so skip the upper-right score block
                    causal = is_self and flags["mask1_pure_causal"]
                    wbf = apool.tile([128, TK, klen], BF, tag="wbf", bufs=3, name="wbf")
                    for qt in range(TK):
                        kw = 128 if (causal and qt == 0) else klen
                        ps = pmid.tile([128, klen], F32, tag="pmid", name="ps_sc")
                        for et in range(DK):
                            nc.tensor.matmul(ps[:, :kw], qbf[:, et, qt * 128:(qt + 1) * 128],
                                             kbf[:, et, :kw], start=(et == 0), stop=(et == DK - 1))
                        if msk is not None:
                            if causal and qt == 1:
                                nc.vector.tensor_add(ps[:, 128:256], ps[:, 128:256],
                                                     msk[:, 1, 128:256])
                            else:
                                nc.vector.tensor_add(ps[:, :kw], ps[:, :kw], msk[:, qt, :kw])
                        nmx = sm.tile([128, 1], F32, tag="nmx", bufs=4, name="nmx")
                        nc.vector.tensor_reduce(out=nmx, in_=ps[:, :kw], axis=AX.X,
                                                op=OP.max, negate=True)
                        den = sm.tile([128, 1], F32, tag="den", bufs=4, name="den")
                        if emit_att:
                            p32 = sm.tile([128, klen], F32, tag="p32", bufs=2, name="p32")
                            nc.scalar.activation(out=p32, in_=ps, func=AF.Exp,
                                                 bias=nmx, accum_out=den)
                            rden = sm.tile([128, 1], F32, tag="rden", bufs=4, name="rden")
                            nc.vector.reciprocal(rden, den)
                            nc.vector.tensor_scalar_mul(out=wbf[:, qt, :], in0=p32, scalar1=rden)
                            att32 = sm.tile([128, klen], F32, tag="att32", bufs=2, name="att32")
                            nc.vector.tensor_scalar_mul(out=att32, in0=p32, scalar1=rden)
                            nc.gpsimd.dma_start(out=attw[h * T + qt * 128:h * T + (qt + 1) * 128, :],
                                                in_=att32)
                        else:
                            pbf = sm.tile([128, klen], BF, tag="pbf", bufs=2, name="pbf")
                            nc.scalar.activation(out=pbf[:, :kw], in_=ps[:, :kw], func=AF.Exp,
                                                 bias=nmx, accum_out=den)
                            rden = sm.tile([128, 1], F32, tag="rden", bufs=4, name="rden")
                            nc.vector.reciprocal(rden, den)
                            nc.vector.tensor_scalar_mul(out=wbf[:, qt, :kw], in0=pbf[:, :kw],
                                                        scalar1=rden)

                    # pipeline: next head's projections run while this head's
                    # softmax results drain through transpose/oT; weights are
                    # prefetched two heads ahead
                    if h + 1 < H:
                        nxt = proj_head(h + 1, wt_next)
                        if h + 2 < H:
                            wt_next = load_head(h + 2)

                    # transpose attention weights: wT[k, q]
                    wtbf = apool.tile([128, kk, T], BF, tag="wtbf", bufs=3, name="wtbf")
                    for qt in range(TK):
                        for k2 in range(kk):
                            if causal and qt == 0 and k2 == 1:
                                continue        # zero block — never read below
                            pt = ptr.tile([128, 128], BF, tag="ptr", name="pt")
                            nc.tensor.transpose(pt, wbf[:, qt, k2 * 128:(k2 + 1) * 128], ident)
                            nc.vector.tensor_copy(wtbf[:, k2, qt * 128:(qt + 1) * 128], pt)

                    # oT[e, q] = v.T @ wT  (+bv via softmax rows summing to 1)
                    for e in range(DK):
                        ps = pmid.tile([128, T], F32, tag="pmid", name="ps_o")
                        esl = slice(e * 128, (e + 1) * 128)
                        if causal:
                            # q-tile 0 sees only k-tile 0; q-tile 1 sees both
                            nc.tensor.matmul(ps[:, 0:128], vbf[:, 0, esl],
                                             wtbf[:, 0, 0:128], start=True, stop=True,
                                             skip_group_check=True)
                            nc.tensor.matmul(ps[:, 128:256], vbf[:, 0, esl],
                                             wtbf[:, 0, 128:256], start=True, stop=False,
                                             skip_group_check=True)
                            nc.tensor.matmul(ps[:, 128:256], vbf[:, 1, esl],
                                             wtbf[:, 1, 128:256], start=False, stop=True,
                                             skip_group_check=True)
                        else:
                            for k2 in range(kk):
                                nc.tensor.matmul(ps, vbf[:, k2, esl],
                                                 wtbf[:, k2, :], start=(k2 == 0),
                                                 stop=(k2 == kk - 1))
                        dst = obf[:, h * DK + e, :]
                        if ub_v:
                            nc.scalar.activation(out=dst, in_=ps, func=AF.Identity,
                                                 bias=bias_col(b, v_base, h * DK + e), scale=1.0)
                        else:
                            nc.scalar.copy(dst, ps)

                # output projection + residual add (+ incremental LN stats)
                wog = []
                for g in range(4):
                    wg = wpool.tile([128, 8, D], BF, tag="wo", bufs=4, name="wo_g")
                    eng = nc.sync if g % 2 == 0 else nc.scalar
                    eng.dma_start(out=wg, in_=wo[g * 1024:(g + 1) * 1024, :]
                                  .rearrange("(j p) n -> p j n", p=128))
                    wog.append(wg)
                for n in range(DK):
                    ps = pmid.tile([128, T], F32, tag="pmid", name="ps_wo")
                    for het in range(HD // 128):
                        nc.tensor.matmul(ps, wog[het // 8][:, het % 8, n * 128:(n + 1) * 128],
                                         obf[:, het, :], start=(het == 0), stop=(het == HD // 128 - 1))
                    if ub_o:
                        asb = sm.tile([128, T], F32, tag="asb", bufs=1, name="asb")
                        nc.scalar.activation(out=asb, in_=ps, func=AF.Identity,
                                             bias=bias_col(b, o_base, n), scale=1.0)
                        nc.vector.tensor_add(res[:, n, :], res[:, n, :], asb)
                    else:
                        nc.vector.tensor_add(res[:, n, :], res[:, n, :], ps)
                    res_stats_chunk(n, pstat)

            def res_stats_chunk(c, pstat):
                """after res[:, c, :] is final for this sublayer, fold its
                column-sum / sum-of-squares into the LN stats psum."""
                nc.gpsimd.tensor_copy(tbf[:, c, :], res[:, c, :])
                nc.gpsimd.tensor_mul(sqbf[:, c, :], tbf[:, c, :], tbf[:, c, :])
                nc.tensor.matmul(pstat[0:1, 0, :], ones_col, tbf[:, c, :],
                                 start=(c == 0), stop=(c == DK - 1), skip_group_check=True)
                nc.tensor.matmul(pstat[0:1, 1, :], ones_col, sqbf[:, c, :],
                                 start=False, stop=(c == DK - 1), skip_group_check=True)

            def layernorm(b, i, pstat):
                """finish LN from accumulated stats: res -> LN(res), xnbf=bf16.
                The mean is broadcast immediately so the (res - mean) subtracts
                overlap the rstd chain (msq/var on DVE, ln/exp on ACT)."""
                me = sm.tile([1, 2, T], F32, tag="srow", bufs=1, name="me")
                nc.vector.tensor_scalar_mul(out=me, in0=pstat, scalar1=1.0 / D)
                mean = me[0:1, 0, :]
                e2 = me[0:1, 1, :]
                bc = pbig.tile([128, 2, T], F32, tag="pbig", name="bc")
                nc.tensor.matmul(bc[:, 1, :], ones_row_f, mean, start=True, stop=True)
                tmps = []
                for c in range(DK):
                    tmp = sm.tile([128, T], F32, tag="lntmp", bufs=4, name="lntmp")
                    nc.vector.tensor_sub(tmp, res[:, c, :], bc[:, 1, :])
                    tmps.append(tmp)
                # rstd chain computed in place over e2: var -> ln -> rstd
                msq = sm.tile([1, T], F32, tag="msq", bufs=1, name="msq")
                nc.vector.tensor_mul(msq, mean, mean)
                nc.vector.tensor_sub(e2, e2, msq)
                nc.scalar.activation(out=e2, in_=e2, func=AF.Ln, bias=eps1)
                rstd = e2
                nc.scalar.activation(out=rstd, in_=rstd, func=AF.Exp, scale=-0.5)
                nc.tensor.matmul(bc[:, 0, :], ones_row_f, rstd, start=False, stop=True,
                                 skip_group_check=True)
                # post-norm: the residual stream itself becomes the LN output
                affine = flags["ln_affine"][b][i]
                for c in range(DK):
                    # write the bf16 matmul input directly (shortest path to the
                    # next projection); refresh the f32 residual lazily from it
                    nc.vector.tensor_mul(xnbf[:, c, :], tmps[c], bc[:, 0, :])
                    if affine:
                        nc.vector.tensor_scalar(out=xnbf[:, c, :], in0=xnbf[:, c, :],
                                                scalar1=ln_col(b, i, 0, c),
                                                scalar2=ln_col(b, i, 1, c),
                                                op0=OP.mult, op1=OP.add)
                    nc.gpsimd.tensor_copy(res[:, c, :], xnbf[:, c, :])

            def ffn_prefetch(b):
                w1k = []
                for kt in range(DK):
                    wt = wpool.tile([128, FF], BF, tag="w1", bufs=5, name="w1_t")
                    eng = nc.sync if kt % 2 == 0 else nc.scalar
                    eng.dma_start(out=wt, in_=w1[b][kt * 128:(kt + 1) * 128, :])
                    w1k.append(wt)
                return w1k

            def ffn(b, pstat, w1k):
                ub1 = flags["bias_b1"][b]
                for n in range(FF // 128):
                    ps = pmid.tile([128, T], F32, tag="pmid", name="ps_h1")
                    for kt in range(DK):
                        nc.tensor.matmul(ps, w1k[kt][:, n * 128:(n + 1) * 128],
                                         xnbf[:, kt, :], start=(kt == 0), stop=(kt == DK - 1))
                    if ub1:
                        nc.scalar.activation(out=h1t[:, n, :], in_=ps, func=AF.Relu,
                                             bias=bias_col(b, _BC_B1, n), scale=1.0)
                    else:
                        nc.scalar.activation(out=h1t[:, n, :], in_=ps, func=AF.Relu)
                w2k = []
                for i in range(8):
                    wt = wpool.tile([128, 2, D], BF, tag="w2", bufs=8, name="w2_t")
                    nc.sync.dma_start(out=wt, in_=w2[b][i * 256:(i + 1) * 256, :]
                                      .rearrange("(j p) n -> p j n", p=128))
                    w2k.append(wt)
                ub2 = flags["bias_b2"][b]
                for n in range(DK):
                    ps = pmid.tile([128, T], F32, tag="pmid", name="ps_h2")
                    for mt in range(FF // 128):
                        nc.tensor.matmul(ps, w2k[mt // 2][:, mt % 2, n * 128:(n + 1) * 128],
                                         h1t[:, mt, :], start=(mt == 0), stop=(mt == FF // 128 - 1))
                    if ub2:
                        asb = sm.tile([128, T], F32, tag="asb", bufs=1, name="asb2")
                        nc.scalar.activation(out=asb, in_=ps, func=AF.Identity,
                                             bias=bias_col(b, _BC_B2, n), scale=1.0)
                        nc.vector.tensor_add(res[:, n, :], res[:, n, :], asb)
                    else:
                        nc.vector.tensor_add(res[:, n, :], res[:, n, :], ps)
                    res_stats_chunk(n, pstat)

            # ---------- the decoder ----------
            def new_pstat():
                return pst.tile([1, 2, T], F32, tag="pst", name="pstat")

            def cross_pre(b):
                # head-0 K/V of cross-attention only needs the encoder states,
                # so emit it before the preceding LN finishes to keep PE busy
                wkv = wpool.tile([128, 2, DK, D], BF, tag="wkv", bufs=4, name="wkv_pre")
                for j, m in enumerate((1, 2)):
                    eng = nc.sync if j % 2 == 0 else nc.scalar
                    eng.dma_start(out=wkv[:, m - 1], in_=wqkv_c[b][m, 0].rearrange(
                        "(kt p) n -> p kt n", p=128))
                wq = wpool.tile([128, DK, D], BF, tag="wq", bufs=3, name="wq_pre")
                nc.scalar.dma_start(out=wq, in_=wqkv_c[b][0, 0].rearrange(
                    "(kt p) n -> p kt n", p=128))
                kbf = apool.tile([128, DK, S], BF, tag="kbf", bufs=2, name="kbf_pre")
                vbf = apool.tile([128, TK, D], BF, tag="vbf", bufs=3, name="vbf_pre")
                ub_k = flags["bias_k_c"][b]
                proj_tn(wkv[:, 0], encst_sb, kbf, 0, b, _BC_K_C, 0, 1.0, ub_k, engine_alt=True)
                v_proj(wkv[:, 1], encst_sb, vbf)
                return (wkv, wq), (kbf, vbf)

            for b in range(NB):
                pstat = new_pstat()
                attention(b, is_self=True, emit_att=False, pstat=pstat)
                pre = cross_pre(b)
                layernorm(b, 0, pstat)
                pstat = new_pstat()
                attention(b, is_self=False, emit_att=(b == 0), pstat=pstat, pre=pre)
                w1k = ffn_prefetch(b)
                layernorm(b, 1, pstat)
                pstat = new_pstat()
                ffn(b, pstat, w1k)
                layernorm(b, 2, pstat)

            # ---------- vocab projection (natural layout) ----------
            # 256-wide psum groups alternating across both psum pools give a
            # 5-slot rotation that decouples MMs / copies / output DMAs
            nch = (VOC + VCH - 1) // VCH
            galt = [0]
            for ci in range(nch):
                c0 = ci * VCH
                cw = min(VCH, VOC - c0)
                wt = wpool.tile([128, DK, VCH], BF, tag="wout", bufs=3, name="wout_t")
                nc.sync.dma_start(out=wt[:, :, :cw],
                                  in_=wout[:, c0:c0 + cw].rearrange("(kt p) n -> p kt n", p=128))
                for qt in range(TK):
                    ysb = sm.tile([128, VCH], DT.float16, tag="ysb", bufs=4, name="ysb")
                    for half in range(2):
                        h0 = half * 256
                        hw = min(256, cw - h0)
                        if hw <= 0:
                            continue
                        galt[0] ^= 1
                        if galt[0]:
                            ps = pmid.tile([128, T], F32, tag="pmid", name="ps_y")
                        else:
                            ps = pbig.tile([128, VCH], F32, tag="pbig", name="ps_y2")
                        for kt in range(DK):
                            nc.tensor.matmul(ps[:, :hw], xnbf[:, kt, qt * 128:(qt + 1) * 128],
                                             wt[:, kt, h0:h0 + hw], start=(kt == 0),
                                             stop=(kt == DK - 1))
                        if half == 0:
                            nc.scalar.copy(ysb[:, h0:h0 + hw], ps[:, :hw])
                        else:
                            nc.vector.tensor_copy(ysb[:, h0:h0 + hw], ps[:, :hw])
                    nc.scalar.dma_start(out=y[qt * 128:(qt + 1) * 128, c0:c0 + cw],
                                        in_=ysb[:, :cw])

    _split_excess_waits(nc)
    return nc


# ---------------------------------------------------------------------------
# host side
# ---------------------------------------------------------------------------

def _pos_enc(seq_len, dim):
    pos = np.arange(seq_len, dtype=np.float32)[:, None]
    den = np.exp(np.arange(0, dim, 2, dtype=np.float32) * (-np.log(10000.0) / dim))
    ang = (pos * den).astype(np.float32)
    pe = np.zeros((seq_len, dim), np.float32)
    pe[:, 0::2] = np.sin(ang)
    pe[:, 1::2] = np.cos(ang)
    return pe


def _nz(a):
    return bool(np.any(np.asarray(a) != 0))


def _compute_flags(params, use_mask2):
    blocks = params["blocks"]
    flags = {
        "use_mask2": use_mask2,
        "bias_q_s": [_nz(bk["a1"]["bq"]) for bk in blocks],
        "bias_k_s": [_nz(bk["a1"]["bk"]) for bk in blocks],
        "bias_v_s": [_nz(bk["a1"]["bv"]) for bk in blocks],
        "bias_o_s": [_nz(bk["a1"]["bo"]) for bk in blocks],
        "bias_q_c": [_nz(bk["a2"]["bq"]) for bk in blocks],
        "bias_k_c": [_nz(bk["a2"]["bk"]) for bk in blocks],
        "bias_v_c": [_nz(bk["a2"]["bv"]) for bk in blocks],
        "bias_o_c": [_nz(bk["a2"]["bo"]) for bk in blocks],
        "bias_b1": [_nz(bk["bias1"]) for bk in blocks],
        "bias_b2": [_nz(bk["bias2"]) for bk in blocks],
        "ln_affine": [[
            _nz(np.asarray(bk[g]) - 1.0) or _nz(bk[bb])
            for g, bb in (("g1", "b1"), ("g2", "b2"), ("g3", "b3"))
        ] for bk in blocks],
    }
    return flags


def _flags_key(flags):
    def freeze(v):
        if isinstance(v, list):
            return tuple(freeze(x) for x in v)
        return v
    return tuple(sorted((k, freeze(v)) for k, v in flags.items()))


def _pack_bias_cols(params):
    """[128, NB, 220] f32 per-partition bias columns (q biases pre-scaled)."""
    out = np.zeros((128, NB, _BC_W), np.float32)

    def put(b, base, vec, scale=1.0):
        v = np.asarray(vec, np.float32).reshape(-1) * scale
        ncols = v.size // 128
        out[:, b, base:base + ncols] = v.reshape(ncols, 128).T

    for b, bk in enumerate(params["blocks"]):
        put(b, _BC_Q_S, bk["a1"]["bq"], RSQ)   # [8,512] -> 32 cols
        put(b, _BC_K_S, bk["a1"]["bk"])
        put(b, _BC_V_S, bk["a1"]["bv"])
        put(b, _BC_O_S, bk["a1"]["bo"])
        put(b, _BC_Q_C, bk["a2"]["bq"], RSQ)
        put(b, _BC_K_C, bk["a2"]["bk"])
        put(b, _BC_V_C, bk["a2"]["bv"])
        put(b, _BC_O_C, bk["a2"]["bo"])
        put(b, _BC_B1, bk["bias1"])
        put(b, _BC_B2, bk["bias2"])
    return out


def _pack_lngb(params):
    out = np.zeros((128, NB, 3, 2, DK), np.float32)
    for b, bk in enumerate(params["blocks"]):
        for i, (g, bb) in enumerate((("g1", "b1"), ("g2", "b2"), ("g3", "b3"))):
            out[:, b, i, 0, :] = np.asarray(bk[g], np.float32).reshape(DK, 128).T
            out[:, b, i, 1, :] = np.asarray(bk[bb], np.float32).reshape(DK, 128).T
    return out


_NC_CACHE = {}


def _get_nc(flags):
    key = _flags_key(flags)
    if key not in _NC_CACHE:
        _NC_CACHE[key] = build_decoder_nc(flags)
    return _NC_CACHE[key]


def _to_bf(a):
    return np.ascontiguousarray(np.asarray(a, np.float32)).astype(BF_NP)


def prepare_in_maps(encoded_source, source_padding, target, params):
    encoded_source = np.asarray(encoded_source, np.float32)
    source_padding = np.asarray(source_padding, np.float32)
    target = np.asarray(target)
    emb = np.asarray(params["emb"], np.float32)

    pe = _pos_enc(T, D)
    tp = (target != V).astype(np.float32)                       # [B,T]
    tril = np.tril(np.ones((T, T), np.float32))

    mask2_full = tp[:, :, None] * source_padding[:, None, :]     # [B,T,S]
    use_mask2 = bool(np.any(mask2_full == 0))
    flags = _compute_flags(params, use_mask2)
    flags["mask1_pure_causal"] = bool(np.all(tp == 1.0))

    shared = {
        "bias_cols": _pack_bias_cols(params),
        "lngb": _pack_lngb(params),
        "wout": _to_bf(params["Wout"]),
    }
    for b, bk in enumerate(params["blocks"]):
        for tag, att in (("s", "a1"), ("c", "a2")):
            w = bk[att]
            shared[f"wqkv_{tag}{b}"] = np.stack([
                _to_bf(w["Wq"]), _to_bf(w["Wk"]), _to_bf(w["Wv"])])  # [3,H,D,D]
            shared[f"wo_{tag}{b}"] = _to_bf(w["Wo"])
        shared[f"w1_{b}"] = _to_bf(bk["W1"])
        shared[f"w2_{b}"] = _to_bf(bk["W2"])

    in_maps = []
    for bidx in range(B):
        x0 = emb[target[bidx]] + pe                              # [T,D] f32
        m1 = tril * np.outer(tp[bidx], tp[bidx])
        im = dict(shared)
        im["x0t"] = np.ascontiguousarray(x0.T.astype(np.float32))
        im["encst"] = np.ascontiguousarray(encoded_source[bidx].T).astype(BF_NP)
        im["mask1"] = np.where(m1 == 0, np.float32(NEG), np.float32(0.0))
        if use_mask2:
            im["mask2"] = np.where(mask2_full[bidx] == 0, np.float32(NEG), np.float32(0.0))
        in_maps.append(im)
    return in_maps, flags


def gather_outputs(results, params):
    y = np.stack([r["y"] for r in results], 0).astype(np.float32)  # [B,T,VOC]
    bout = np.asarray(params["bout"], np.float32)
    if np.any(bout != 0):
        y = y + bout[None, None, :]
    att = np.stack([r["attw"].reshape(H, T, S) for r in results], 0)
    return y, att


class _Runner:
    """Cached jitted SPMD executor over jax.devices()[:8] with repeat-timing
    support (outputs recycled as donated buffers)."""

    def __init__(self, nc):
        import jax
        from concourse import bass2jax as B2J
        from jax.experimental.shard_map import shard_map
        from jax.sharding import Mesh, PartitionSpec, NamedSharding

        B2J.install_neuronx_cc_hook()
        self.nc = nc
        partition_name = nc.partition_id_tensor.name if nc.partition_id_tensor else None
        in_names, out_names, out_avals, zero_outs = [], [], [], []
        for alloc in nc.m.functions[0].allocations:
            if not isinstance(alloc, mybir.MemoryLocationSet):
                continue
            name = alloc.memorylocations[0].name
            if alloc.kind == "ExternalInput":
                if name != partition_name:
                    in_names.append(name)
            elif alloc.kind == "ExternalOutput":
                out_names.append(name)
                shape = tuple(alloc.tensor_shape)
                dtype = mybir.dt.np(alloc.dtype)
                out_avals.append(jax.core.ShapedArray(shape, dtype))
                zero_outs.append(np.zeros(shape, dtype))
        self.in_names = list(in_names)
        self.out_names = out_names
        self.zero_outs = zero_outs
        n_params = len(in_names)
        n_outs = len(out_avals)
        all_in = in_names + out_names + ([partition_name] if partition_name else [])

        def _body(*args):
            operands = list(args)
            if partition_name is not None:
                operands.append(B2J.partition_id_tensor())
            outs = B2J._bass_exec_p.bind(
                *operands,
                out_avals=tuple(out_avals),
                in_names=tuple(all_in),
                out_names=tuple(out_names),
                lowering_input_output_aliases=(),
                sim_require_finite=True,
                sim_require_nnan=True,
                nc=nc,
            )
            return tuple(outs)

        devices = jax.devices()[:B]
        assert len(devices) == B
        self.mesh = Mesh(np.asarray(devices), ("core",))
        self.spec = PartitionSpec("core")
        self.sharding = NamedSharding(self.mesh, self.spec)
        in_specs = (self.spec,) * (n_params + n_outs)
        out_specs = (self.spec,) * n_outs
        donate = tuple(range(n_params, n_params + n_outs))
        self.fn = jax.jit(
            shard_map(_body, mesh=self.mesh, in_specs=in_specs,
                      out_specs=out_specs, check_rep=False),
            donate_argnums=donate, keep_unused=True,
        )
        self._dev_in = None
        self._jax = jax

    def put_inputs(self, in_maps):
        jax = self._jax
        concat = [np.concatenate([np.asarray(m[n]) for m in in_maps], axis=0)
                  for n in self.in_names]
        self._dev_in = [jax.device_put(a, self.sharding) for a in concat]

    def _zeros_dev(self):
        jax = self._jax
        import jax.numpy as jnp
        if not hasattr(self, "_zfn"):
            shapes = [((B * z.shape[0],) + z.shape[1:], z.dtype) for z in self.zero_outs]
            self._zfn = jax.jit(
                lambda: tuple(jnp.zeros(s, d) for s, d in shapes),
                out_shardings=tuple(self.sharding for _ in shapes))
        return list(self._zfn())

    def run(self):
        outs = self.fn(*self._dev_in, *self._zeros_dev())
        self._jax.block_until_ready(outs)
        host = [np.asarray(o) for o in outs]
        results = []
        for c in range(B):
            r = {}
            for i, name in enumerate(self.out_names):
                r[name] = host[i].reshape(B, host[i].shape[0] // B, *host[i].shape[1:])[c]
            results.append(r)
        return results

    def time(self, reps=8):
        import time as _t
        outs = self.fn(*self._dev_in, *self._zeros_dev())
        self._jax.block_until_ready(outs)
        best = float("inf")
        for _ in range(reps):
            t0 = _t.perf_counter()
            outs = self.fn(*self._dev_in, *outs)
            self._jax.block_until_ready(outs)
            best = min(best, _t.perf_counter() - t0)
        return best * 1e9

    def _run_k(self, k):
        import time as _t
        o = self._zeros_dev()
        t0 = _t.perf_counter()
        for _ in range(k):
            o = self.fn(*self._dev_in, *o)
        self._jax.block_until_ready(o)
        return _t.perf_counter() - t0

    def time_slope(self, k1=4, k2=24, reps=3):
        """per-execution time from the marginal cost of extra chained runs;
        removes the axon round-trip latency (~80ms) from the estimate."""
        outs = self.fn(*self._dev_in, *self._zeros_dev())
        self._jax.block_until_ready(outs)
        del outs
        best = float("inf")
        for _ in range(reps):
            t1 = self._run_k(k1)
            t2 = self._run_k(k2)
            best = min(best, (t2 - t1) / (k2 - k1))
        return best * 1e9


_RUNNER_CACHE = {}


def _get_runner(flags):
    key = _flags_key(flags)
    if key not in _RUNNER_CACHE:
        _RUNNER_CACHE[key] = _Runner(_get_nc(flags))
    return _RUNNER_CACHE[key]


def kernel(encoded_source, source_padding, target, params):
    in_maps, flags = prepare_in_maps(encoded_source, source_padding, target, params)
    runner = _get_runner(flags)
    runner.put_inputs(in_maps)
    return gather_outputs(runner.run(), params)


def time_kernel(encoded_source, source_padding, target, params, reps=8):
    in_maps, flags = prepare_in_maps(encoded_source, source_padding, target, params)
    runner = _get_runner(flags)
    runner.put_inputs(in_maps)
    return runner.time_slope()


# revision 37
# speedup vs baseline: 1.3911x; 1.3911x over previous
"""Trainium2 Bass kernel for a 4-block transformer decoder (nn_Decoder).

Strategy: data-parallel over batch across 8 NeuronCores (1 batch element per
core), no collectives. Per core the whole decoder runs on [T=256, D=512]
activations kept feature-major ("TN layout": features on SBUF partitions,
tokens on the free dim), so every linear layer consumes weights as the
stationary matmul operand directly in their natural [in, out] layout.
Weights are converted to bf16 on the host (halves HBM traffic; fp32 matmul
on TRN2 runs at 1/4 rate); accumulation stays fp32 in PSUM and the residual
stream / softmax / layernorm statistics stay fp32.
"""

import numpy as np
import ml_dtypes

import bass_rust
import concourse.bass as bass
import concourse.mybir as mybir
from concourse.tile import TileContext
from concourse.masks import make_identity
from concourse.bass_utils import run_bass_kernel_spmd

DT = mybir.dt
BF = DT.bfloat16
F32 = DT.float32
AF = mybir.ActivationFunctionType
AX = mybir.AxisListType
OP = mybir.AluOpType
BF_NP = ml_dtypes.bfloat16

# Model dims (fixed by the problem)
V = 32000
D = 512
H = 8
NB = 4
B = 8
S = 256
T = 256
DK = D // 128          # 4 k-tiles over the model dim
TK = T // 128          # 2 token tiles
HD = H * D             # 4096 concat-head dim
FF = 4 * D             # 2048
VOC = V + 1            # 32001
VCH = 512              # vocab free-dim chunk
NEG = -1.0e30
RSQ = 1.0 / float(np.sqrt(np.float32(D)))

# bias-column layout inside bias_cols[:, block, col]
_BC_Q_S, _BC_K_S, _BC_V_S, _BC_O_S = 0, 32, 64, 96
_BC_Q_C, _BC_K_C, _BC_V_C, _BC_O_C = 100, 132, 164, 196
_BC_B1, _BC_B2 = 200, 216
_BC_W = 220


def _split_excess_waits(nc, max_waits=1):
    """walrus in this container encodes at most one semaphore wait per
    instruction; move extra waits onto same-engine carrier nops."""
    for bb in nc.main_func.blocks:
        insts = bb.instructions
        def nwaits(ins):
            si = ins.sync_info
            return len(si.on_wait) if si is not None else 0
        if not any(nwaits(i) > max_waits for i in insts):
            continue
        new_list = []
        for ins in list(insts):
            si0 = ins.sync_info
            waits = list(si0.on_wait) if si0 is not None else []
            if len(waits) > max_waits:
                excess = waits[: len(waits) - max_waits]
                keep = waits[len(waits) - max_waits:]
                eng = nc.engines[ins.engine]
                for i in range(0, len(excess), max_waits):
                    chunk = excess[i:i + max_waits]
                    carrier = eng.nop(nofuse=True, hint="wait_split")
                    cins = carrier.ins
                    cur = nc.cur_bb.bb.instructions
                    assert cur[-1].name == cins.name
                    cur.pop()
                    cins.sync_info = bass_rust.SyncInfo(on_wait=chunk, on_update=[])
                    new_list.append(cins)
                si0.on_wait = keep
            new_list.append(ins)
        insts[:] = new_list


def build_decoder_nc(flags):
    """flags: dict with booleans use_mask2, bias_* (see _compute_flags)."""
    nc = bass.Bass()

    x0t = nc.declare_dram_parameter("x0t", [D, T], F32, isOutput=False)
    encst = nc.declare_dram_parameter("encst", [D, S], BF, isOutput=False)
    mask1 = nc.declare_dram_parameter("mask1", [T, T], F32, isOutput=False)
    if flags["use_mask2"]:
        mask2 = nc.declare_dram_parameter("mask2", [T, S], F32, isOutput=False)
    bias_cols = nc.declare_dram_parameter("bias_cols", [128, NB, _BC_W], F32, isOutput=False)
    lngb = nc.declare_dram_parameter("lngb", [128, NB, 3, 2, DK], F32, isOutput=False)
    wqkv_s = [nc.declare_dram_parameter(f"wqkv_s{b}", [3, H, D, D], BF, isOutput=False) for b in range(NB)]
    wqkv_c = [nc.declare_dram_parameter(f"wqkv_c{b}", [3, H, D, D], BF, isOutput=False) for b in range(NB)]
    wo_s = [nc.declare_dram_parameter(f"wo_s{b}", [HD, D], BF, isOutput=False) for b in range(NB)]
    wo_c = [nc.declare_dram_parameter(f"wo_c{b}", [HD, D], BF, isOutput=False) for b in range(NB)]
    w1 = [nc.declare_dram_parameter(f"w1_{b}", [D, FF], BF, isOutput=False) for b in range(NB)]
    w2 = [nc.declare_dram_parameter(f"w2_{b}", [FF, D], BF, isOutput=False) for b in range(NB)]
    wout = nc.declare_dram_parameter("wout", [D, VOC], BF, isOutput=False)

    y = nc.declare_dram_parameter("y", [T, VOC], DT.float16, isOutput=True)
    attw = nc.declare_dram_parameter("attw", [H * T, S], F32, isOutput=True)

    with TileContext(nc) as tc:
        with (
            tc.tile_pool(name="const", bufs=1) as const,
            tc.tile_pool(name="persist", bufs=1) as persist,
            tc.tile_pool(name="wpool", bufs=2) as wpool,
            tc.tile_pool(name="apool", bufs=2) as apool,
            tc.tile_pool(name="sm", bufs=3) as sm,
            tc.tile_pool(name="pmid", bufs=3, space="PSUM") as pmid,
            tc.tile_pool(name="pbig", bufs=2, space="PSUM") as pbig,
            tc.tile_pool(name="ptr", bufs=2, space="PSUM") as ptr,
            tc.tile_pool(name="pst", bufs=1, space="PSUM") as pst,
        ):
            # ---- constants ----
            ident = const.tile([128, 128], BF)
            make_identity(nc, ident)
            ones_col = const.tile([128, 1], BF)
            nc.vector.memset(ones_col, 1.0)
            ones_row_f = const.tile([1, 128], F32)
            nc.vector.memset(ones_row_f, 1.0)
            ones_row_b = const.tile([1, 128], BF)
            nc.vector.memset(ones_row_b, 1.0)
            eps1 = const.tile([1, 1], F32)
            nc.vector.memset(eps1, 1e-5)

            # ---- persistent state ----
            res = persist.tile([128, DK, T], F32)       # residual stream (xT)
            xnbf = persist.tile([128, DK, T], BF)       # current matmul input (xT, bf16)
            encst_sb = persist.tile([128, DK, S], BF)
            obf = persist.tile([128, H * DK, T], BF)    # concat head outputs (oT)
            h1t = persist.tile([128, FF // 128, T], BF)
            tbf = persist.tile([128, DK, T], BF)        # LN scratch: bf16 copy of res
            sqbf = persist.tile([128, DK, T], BF)       # LN scratch: squares
            mask1_sb = persist.tile([128, TK, T], F32)
            if flags["use_mask2"]:
                mask2_sb = persist.tile([128, TK, S], F32)
            bias_sb = persist.tile([128, NB, _BC_W], F32)
            lngb_sb = persist.tile([128, NB, 3, 2, DK], F32)

            # ---- input DMA ----
            nc.sync.dma_start(out=res, in_=x0t.rearrange("(kt p) t -> p kt t", p=128))
            nc.sync.dma_start(out=encst_sb, in_=encst.rearrange("(kt p) t -> p kt t", p=128))
            nc.sync.dma_start(out=mask1_sb, in_=mask1.rearrange("(qt p) k -> p qt k", p=128))
            if flags["use_mask2"]:
                nc.sync.dma_start(out=mask2_sb, in_=mask2.rearrange("(qt p) k -> p qt k", p=128))
            nc.sync.dma_start(out=bias_sb, in_=bias_cols[:, :, :])
            nc.sync.dma_start(out=lngb_sb, in_=lngb[:, :, :, :, :])

            for c in range(DK):
                nc.gpsimd.tensor_copy(xnbf[:, c, :], res[:, c, :])

            def bias_col(b, base, idx):
                return bias_sb[:, b, base + idx:base + idx + 1]

            def ln_col(b, i, gb, c):
                return lngb_sb[:, b, i, gb, c:c + 1]

            # ---------- building blocks ----------

            def proj_tn(wtile, x_in, out_tile, out_idx, b, bias_base, h,
                        scale, use_bias, engine_alt=False):
                """TN projection: out[:, out_idx+n, :] = W.T @ x (+bias) for n in 0..DK-1."""
                for n in range(DK):
                    ps = pmid.tile([128, T], F32, tag="pmid", name="ps_proj")
                    for kt in range(DK):
                        nc.tensor.matmul(ps, wtile[:, kt, n * 128:(n + 1) * 128],
                                         x_in[:, kt, :], start=(kt == 0), stop=(kt == DK - 1))
                    dst = out_tile[:, out_idx + n, :]
                    if use_bias:
                        bcol = bias_col(b, bias_base, (h * DK + n) if h is not None else n)
                        nc.scalar.activation(out=dst, in_=ps, func=AF.Identity,
                                             bias=bcol, scale=scale)
                    elif scale != 1.0:
                        nc.scalar.mul(dst, ps, scale)
                    elif engine_alt:
                        nc.vector.tensor_copy(dst, ps)
                    else:
                        nc.scalar.copy(dst, ps)

            def v_proj(wtile, x_in, vout):
                """natural-layout value projection: v[t, e] (bias folded in at oT)."""
                for tk in range(TK):
                    ps = pbig.tile([128, D], F32, tag="pbig", name="ps_v")
                    for kt in range(DK):
                        nc.tensor.matmul(ps, x_in[:, kt, tk * 128:(tk + 1) * 128],
                                         wtile[:, kt, :], start=(kt == 0), stop=(kt == DK - 1))
                    nc.vector.tensor_copy(vout[:, tk, :], ps)

            def attention(b, is_self, emit_att, pstat, pre=None):
                """one MHA sublayer; reads xnbf (queries) and for cross the
                encoder states; accumulates output into res via Wo.

                Head loop is software-pipelined: head h+1's K/V/Q projections
                are emitted between head h's softmax and its transpose/oT so
                the PE stays busy during the softmax latency. K/V are
                projected before Q so that for cross-attention the (LN-
                independent) K/V matmuls can run while the preceding
                layernorm finishes."""
                wqkv = wqkv_s[b] if is_self else wqkv_c[b]
                wo = wo_s[b] if is_self else wo_c[b]
                kvx = xnbf if is_self else encst_sb
                klen = T if is_self else S
                kk = klen // 128
                q_base = _BC_Q_S if is_self else _BC_Q_C
                k_base = _BC_K_S if is_self else _BC_K_C
                v_base = _BC_V_S if is_self else _BC_V_C
                o_base = _BC_O_S if is_self else _BC_O_C
                ub_q = flags["bias_q_s" if is_self else "bias_q_c"][b]
                ub_k = flags["bias_k_s" if is_self else "bias_k_c"][b]
                ub_v = flags["bias_v_s" if is_self else "bias_v_c"][b]
                ub_o = flags["bias_o_s" if is_self else "bias_o_c"][b]
                msk = None
                if is_self:
                    msk = mask1_sb
                elif flags["use_mask2"]:
                    msk = mask2_sb

                def load_head(h):
                    # split K/V (used first) from Q so their prefetch depths
                    # decouple
                    wkv = wpool.tile([128, 2, DK, D], BF, tag="wkv", bufs=4, name="wkv")
                    for j, m in enumerate((1, 2)):
                        eng = nc.sync if (h + j) % 2 == 0 else nc.scalar
                        eng.dma_start(out=wkv[:, m - 1], in_=wqkv[m, h].rearrange(
                            "(kt p) n -> p kt n", p=128))
                    wq = wpool.tile([128, DK, D], BF, tag="wq", bufs=3, name="wq")
                    eng = nc.sync if h % 2 == 0 else nc.scalar
                    eng.dma_start(out=wq, in_=wqkv[0, h].rearrange(
                        "(kt p) n -> p kt n", p=128))
                    return wkv, wq

                def proj_kv(h, wkv):
                    kbf = apool.tile([128, DK, S], BF, tag="kbf", bufs=2, name="kbf")
                    vbf = apool.tile([128, kk, D], BF, tag="vbf", bufs=3, name="vbf")
                    proj_tn(wkv[:, 0], kvx, kbf, 0, b, k_base, h, 1.0, ub_k, engine_alt=True)
                    v_proj(wkv[:, 1], kvx, vbf)
                    return kbf, vbf

                def proj_head(h, wt, kv=None):
                    wkv, wq = wt
                    kbf, vbf = kv if kv is not None else proj_kv(h, wkv)
                    qbf = apool.tile([128, DK, T], BF, tag="qbf", bufs=2, name="qbf")
                    proj_tn(wq, xnbf, qbf, 0, b, q_base, h, RSQ, ub_q)
                    return qbf, kbf, vbf

                if pre is not None:
                    wt, kv0 = pre
                    nxt = proj_head(0, wt, kv0)
                else:
                    wt = load_head(0)
                    nxt = proj_head(0, wt)
                wt_next = load_head(1)
                for h in range(H):
                    qbf, kbf, vbf = nxt
                    # pure causal self-attention: queries in tile 0 only attend
                    # keys in tile 0, so skip the upper-right score block
                    causal = is_self and flags["mask1_pure_causal"]
                    wbf = apool.tile([128, TK, klen], BF, tag="wbf", bufs=3, name="wbf")
                    for qt in range(TK):
                        kw = 128 if (causal and qt == 0) else klen
                        ps = pmid.tile([128, klen], F32, tag="pmid", name="ps_sc")
                        for et in range(DK):
                            nc.tensor.matmul(ps[:, :kw], qbf[:, et, qt * 128:(qt + 1) * 128],
                                             kbf[:, et, :kw], start=(et == 0), stop=(et == DK - 1))
                        if msk is not None:
                            if causal and qt == 1:
                                nc.vector.tensor_add(ps[:, 128:256], ps[:, 128:256],
                                                     msk[:, 1, 128:256])
                            else:
                                nc.vector.tensor_add(ps[:, :kw], ps[:, :kw], msk[:, qt, :kw])
                        nmx = sm.tile([128, 1], F32, tag="nmx", bufs=4, name="nmx")
                        nc.vector.tensor_reduce(out=nmx, in_=ps[:, :kw], axis=AX.X,
                                                op=OP.max, negate=True)
                        den = sm.tile([128, 1], F32, tag="den", bufs=4, name="den")
                        if emit_att:
                            p32 = sm.tile([128, klen], F32, tag="p32", bufs=2, name="p32")
                            nc.scalar.activation(out=p32, in_=ps, func=AF.Exp,
                                                 bias=nmx, accum_out=den)
                            rden = sm.tile([128, 1], F32, tag="rden", bufs=4, name="rden")
                            nc.vector.reciprocal(rden, den)
                            nc.vector.tensor_scalar_mul(out=wbf[:, qt, :], in0=p32, scalar1=rden)
                            att32 = sm.tile([128, klen], F32, tag="att32", bufs=2, name="att32")
                            nc.vector.tensor_scalar_mul(out=att32, in0=p32, scalar1=rden)
                            nc.gpsimd.dma_start(out=attw[h * T + qt * 128:h * T + (qt + 1) * 128, :],
                                                in_=att32)
                        else:
                            pbf = sm.tile([128, klen], BF, tag="pbf", bufs=2, name="pbf")
                            nc.scalar.activation(out=pbf[:, :kw], in_=ps[:, :kw], func=AF.Exp,
                                                 bias=nmx, accum_out=den)
                            rden = sm.tile([128, 1], F32, tag="rden", bufs=4, name="rden")
                            nc.vector.reciprocal(rden, den)
                            nc.vector.tensor_scalar_mul(out=wbf[:, qt, :kw], in0=pbf[:, :kw],
                                                        scalar1=rden)

                    # pipeline: next head's projections run while this head's
                    # softmax results drain through transpose/oT; weights are
                    # prefetched two heads ahead
                    if h + 1 < H:
                        nxt = proj_head(h + 1, wt_next)
                        if h + 2 < H:
                            wt_next = load_head(h + 2)

                    # transpose attention weights: wT[k, q]
                    wtbf = apool.tile([128, kk, T], BF, tag="wtbf", bufs=3, name="wtbf")
                    for qt in range(TK):
                        for k2 in range(kk):
                            if causal and qt == 0 and k2 == 1:
                                continue        # zero block — never read below
                            pt = ptr.tile([128, 128], BF, tag="ptr", name="pt")
                            nc.tensor.transpose(pt, wbf[:, qt, k2 * 128:(k2 + 1) * 128], ident)
                            nc.vector.tensor_copy(wtbf[:, k2, qt * 128:(qt + 1) * 128], pt)

                    # oT[e, q] = v.T @ wT  (+bv via softmax rows summing to 1)
                    for e in range(DK):
                        ps = pmid.tile([128, T], F32, tag="pmid", name="ps_o")
                        esl = slice(e * 128, (e + 1) * 128)
                        if causal:
                            # q-tile 0 sees only k-tile 0; q-tile 1 sees both
                            nc.tensor.matmul(ps[:, 0:128], vbf[:, 0, esl],
                                             wtbf[:, 0, 0:128], start=True, stop=True,
                                             skip_group_check=True)
                            nc.tensor.matmul(ps[:, 128:256], vbf[:, 0, esl],
                                             wtbf[:, 0, 128:256], start=True, stop=False,
                                             skip_group_check=True)
                            nc.tensor.matmul(ps[:, 128:256], vbf[:, 1, esl],
                                             wtbf[:, 1, 128:256], start=False, stop=True,
                                             skip_group_check=True)
                        else:
                            for k2 in range(kk):
                                nc.tensor.matmul(ps, vbf[:, k2, esl],
                                                 wtbf[:, k2, :], start=(k2 == 0),
                                                 stop=(k2 == kk - 1))
                        dst = obf[:, h * DK + e, :]
                        if ub_v:
                            nc.scalar.activation(out=dst, in_=ps, func=AF.Identity,
                                                 bias=bias_col(b, v_base, h * DK + e), scale=1.0)
                        else:
                            nc.scalar.copy(dst, ps)

                # output projection + residual add (+ incremental LN stats)
                wog = []
                for g in range(4):
                    wg = wpool.tile([128, 8, D], BF, tag="wo", bufs=4, name="wo_g")
                    eng = nc.sync if g % 2 == 0 else nc.scalar
                    eng.dma_start(out=wg, in_=wo[g * 1024:(g + 1) * 1024, :]
                                  .rearrange("(j p) n -> p j n", p=128))
                    wog.append(wg)
                for n in range(DK):
                    ps = pmid.tile([128, T], F32, tag="pmid", name="ps_wo")
                    for het in range(HD // 128):
                        nc.tensor.matmul(ps, wog[het // 8][:, het % 8, n * 128:(n + 1) * 128],
                                         obf[:, het, :], start=(het == 0), stop=(het == HD // 128 - 1))
                    if ub_o:
                        asb = sm.tile([128, T], F32, tag="asb", bufs=1, name="asb")
                        nc.scalar.activation(out=asb, in_=ps, func=AF.Identity,
                                             bias=bias_col(b, o_base, n), scale=1.0)
                        nc.vector.tensor_add(res[:, n, :], res[:, n, :], asb)
                    else:
                        nc.vector.tensor_add(res[:, n, :], res[:, n, :], ps)
                    res_stats_chunk(n, pstat)

            def res_stats_chunk(c, pstat):
                """after res[:, c, :] is final for this sublayer, fold its
                column-sum / sum-of-squares into the LN stats psum."""
                nc.gpsimd.tensor_copy(tbf[:, c, :], res[:, c, :])
                nc.gpsimd.tensor_mul(sqbf[:, c, :], tbf[:, c, :], tbf[:, c, :])
                nc.tensor.matmul(pstat[0:1, 0, :], ones_col, tbf[:, c, :],
                                 start=(c == 0), stop=(c == DK - 1), skip_group_check=True)
                nc.tensor.matmul(pstat[0:1, 1, :], ones_col, sqbf[:, c, :],
                                 start=False, stop=(c == DK - 1), skip_group_check=True)

            def layernorm(b, i, pstat):
                """finish LN from accumulated stats: res -> LN(res), xnbf=bf16.
                The mean is broadcast immediately so the (res - mean) subtracts
                overlap the rstd chain (msq/var on DVE, ln/exp on ACT)."""
                me = sm.tile([1, 2, T], F32, tag="srow", bufs=1, name="me")
                nc.vector.tensor_scalar_mul(out=me, in0=pstat, scalar1=1.0 / D)
                mean = me[0:1, 0, :]
                e2 = me[0:1, 1, :]
                bc = pbig.tile([128, 2, T], F32, tag="pbig", name="bc")
                nc.tensor.matmul(bc[:, 1, :], ones_row_f, mean, start=True, stop=True)
                tmps = []
                for c in range(DK):
                    tmp = sm.tile([128, T], F32, tag="lntmp", bufs=4, name="lntmp")
                    nc.vector.tensor_sub(tmp, res[:, c, :], bc[:, 1, :])
                    tmps.append(tmp)
                # rstd chain computed in place over e2: var -> ln -> rstd
                msq = sm.tile([1, T], F32, tag="msq", bufs=1, name="msq")
                nc.vector.tensor_mul(msq, mean, mean)
                nc.vector.tensor_sub(e2, e2, msq)
                nc.scalar.activation(out=e2, in_=e2, func=AF.Ln, bias=eps1)
                rstd = e2
                nc.scalar.activation(out=rstd, in_=rstd, func=AF.Exp, scale=-0.5)
                nc.tensor.matmul(bc[:, 0, :], ones_row_f, rstd, start=False, stop=True,
                                 skip_group_check=True)
                # post-norm: the residual stream itself becomes the LN output
                affine = flags["ln_affine"][b][i]
                for c in range(DK):
                    # write the bf16 matmul input directly (shortest path to the
                    # next projection); refresh the f32 residual lazily from it
                    nc.vector.tensor_mul(xnbf[:, c, :], tmps[c], bc[:, 0, :])
                    if affine:
                        nc.vector.tensor_scalar(out=xnbf[:, c, :], in0=xnbf[:, c, :],
                                                scalar1=ln_col(b, i, 0, c),
                                                scalar2=ln_col(b, i, 1, c),
                                                op0=OP.mult, op1=OP.add)
                    nc.gpsimd.tensor_copy(res[:, c, :], xnbf[:, c, :])

            def ffn_prefetch(b):
                w1k = []
                for kt in range(DK):
                    wt = wpool.tile([128, FF], BF, tag="w1", bufs=5, name="w1_t")
                    eng = nc.sync if kt % 2 == 0 else nc.scalar
                    eng.dma_start(out=wt, in_=w1[b][kt * 128:(kt + 1) * 128, :])
                    w1k.append(wt)
                return w1k

            def ffn(b, pstat, w1k):
                ub1 = flags["bias_b1"][b]
                for n in range(FF // 128):
                    ps = pmid.tile([128, T], F32, tag="pmid", name="ps_h1")
                    for kt in range(DK):
                        nc.tensor.matmul(ps, w1k[kt][:, n * 128:(n + 1) * 128],
                                         xnbf[:, kt, :], start=(kt == 0), stop=(kt == DK - 1))
                    if ub1:
                        nc.scalar.activation(out=h1t[:, n, :], in_=ps, func=AF.Relu,
                                             bias=bias_col(b, _BC_B1, n), scale=1.0)
                    else:
                        nc.scalar.activation(out=h1t[:, n, :], in_=ps, func=AF.Relu)
                w2k = []
                for i in range(8):
                    wt = wpool.tile([128, 2, D], BF, tag="w2", bufs=8, name="w2_t")
                    nc.sync.dma_start(out=wt, in_=w2[b][i * 256:(i + 1) * 256, :]
                                      .rearrange("(j p) n -> p j n", p=128))
                    w2k.append(wt)
                ub2 = flags["bias_b2"][b]
                for n in range(DK):
                    ps = pmid.tile([128, T], F32, tag="pmid", name="ps_h2")
                    for mt in range(FF // 128):
                        nc.tensor.matmul(ps, w2k[mt // 2][:, mt % 2, n * 128:(n + 1) * 128],
                                         h1t[:, mt, :], start=(mt == 0), stop=(mt == FF // 128 - 1))
                    if ub2:
                        asb = sm.tile([128, T], F32, tag="asb", bufs=1, name="asb2")
                        nc.scalar.activation(out=asb, in_=ps, func=AF.Identity,
                                             bias=bias_col(b, _BC_B2, n), scale=1.0)
                        nc.vector.tensor_add(res[:, n, :], res[:, n, :], asb)
                    else:
                        nc.vector.tensor_add(res[:, n, :], res[:, n, :], ps)
                    res_stats_chunk(n, pstat)

            # ---------- the decoder ----------
            def new_pstat():
                return pst.tile([1, 2, T], F32, tag="pst", name="pstat")

            def cross_pre(b):
                # head-0 K/V of cross-attention only needs the encoder states,
                # so emit it before the preceding LN finishes to keep PE busy
                wkv = wpool.tile([128, 2, DK, D], BF, tag="wkv", bufs=4, name="wkv_pre")
                for j, m in enumerate((1, 2)):
                    eng = nc.sync if j % 2 == 0 else nc.scalar
                    eng.dma_start(out=wkv[:, m - 1], in_=wqkv_c[b][m, 0].rearrange(
                        "(kt p) n -> p kt n", p=128))
                wq = wpool.tile([128, DK, D], BF, tag="wq", bufs=3, name="wq_pre")
                nc.scalar.dma_start(out=wq, in_=wqkv_c[b][0, 0].rearrange(
                    "(kt p) n -> p kt n", p=128))
                kbf = apool.tile([128, DK, S], BF, tag="kbf", bufs=2, name="kbf_pre")
                vbf = apool.tile([128, TK, D], BF, tag="vbf", bufs=3, name="vbf_pre")
                ub_k = flags["bias_k_c"][b]
                proj_tn(wkv[:, 0], encst_sb, kbf, 0, b, _BC_K_C, 0, 1.0, ub_k, engine_alt=True)
                v_proj(wkv[:, 1], encst_sb, vbf)
                return (wkv, wq), (kbf, vbf)

            for b in range(NB):
                pstat = new_pstat()
                attention(b, is_self=True, emit_att=False, pstat=pstat)
                pre = cross_pre(b)
                layernorm(b, 0, pstat)
                pstat = new_pstat()
                attention(b, is_self=False, emit_att=(b == 0), pstat=pstat, pre=pre)
                w1k = ffn_prefetch(b)
                layernorm(b, 1, pstat)
                pstat = new_pstat()
                ffn(b, pstat, w1k)
                layernorm(b, 2, pstat)

            # ---------- vocab projection (natural layout) ----------
            # 256-wide psum groups alternating across both psum pools give a
            # 5-slot rotation that decouples MMs / copies / output DMAs
            nch = (VOC + VCH - 1) // VCH
            galt = [0]
            for ci in range(nch):
                c0 = ci * VCH
                cw = min(VCH, VOC - c0)
                wt = wpool.tile([128, DK, VCH], BF, tag="wout", bufs=3, name="wout_t")
                nc.sync.dma_start(out=wt[:, :, :cw],
                                  in_=wout[:, c0:c0 + cw].rearrange("(kt p) n -> p kt n", p=128))
                for qt in range(TK):
                    ysb = sm.tile([128, VCH], DT.float16, tag="ysb", bufs=4, name="ysb")
                    for half in range(2):
                        h0 = half * 256
                        hw = min(256, cw - h0)
                        if hw <= 0:
                            continue
                        galt[0] ^= 1
                        if galt[0]:
                            ps = pmid.tile([128, T], F32, tag="pmid", name="ps_y")
                        else:
                            ps = pbig.tile([128, VCH], F32, tag="pbig", name="ps_y2")
                        for kt in range(DK):
                            nc.tensor.matmul(ps[:, :hw], xnbf[:, kt, qt * 128:(qt + 1) * 128],
                                             wt[:, kt, h0:h0 + hw], start=(kt == 0),
                                             stop=(kt == DK - 1))
                        if half == 0:
                            nc.scalar.copy(ysb[:, h0:h0 + hw], ps[:, :hw])
                        else:
                            nc.vector.tensor_copy(ysb[:, h0:h0 + hw], ps[:, :hw])
                    nc.scalar.dma_start(out=y[qt * 128:(qt + 1) * 128, c0:c0 + cw],
                                        in_=ysb[:, :cw])

    _split_excess_waits(nc)
    return nc


# ---------------------------------------------------------------------------
# host side
# ---------------------------------------------------------------------------

def _pos_enc(seq_len, dim):
    pos = np.arange(seq_len, dtype=np.float32)[:, None]
    den = np.exp(np.arange(0, dim, 2, dtype=np.float32) * (-np.log(10000.0) / dim))
    ang = (pos * den).astype(np.float32)
    pe = np.zeros((seq_len, dim), np.float32)
    pe[:, 0::2] = np.sin(ang)
    pe[:, 1::2] = np.cos(ang)
    return pe


def _nz(a):
    return bool(np.any(np.asarray(a) != 0))


def _compute_flags(params, use_mask2):
    blocks = params["blocks"]
    flags = {
        "use_mask2": use_mask2,
        "bias_q_s": [_nz(bk["a1"]["bq"]) for bk in blocks],
        "bias_k_s": [_nz(bk["a1"]["bk"]) for bk in blocks],
        "bias_v_s": [_nz(bk["a1"]["bv"]) for bk in blocks],
        "bias_o_s": [_nz(bk["a1"]["bo"]) for bk in blocks],
        "bias_q_c": [_nz(bk["a2"]["bq"]) for bk in blocks],
        "bias_k_c": [_nz(bk["a2"]["bk"]) for bk in blocks],
        "bias_v_c": [_nz(bk["a2"]["bv"]) for bk in blocks],
        "bias_o_c": [_nz(bk["a2"]["bo"]) for bk in blocks],
        "bias_b1": [_nz(bk["bias1"]) for bk in blocks],
        "bias_b2": [_nz(bk["bias2"]) for bk in blocks],
        "ln_affine": [[
            _nz(np.asarray(bk[g]) - 1.0) or _nz(bk[bb])
            for g, bb in (("g1", "b1"), ("g2", "b2"), ("g3", "b3"))
        ] for bk in blocks],
    }
    return flags


def _flags_key(flags):
    def freeze(v):
        if isinstance(v, list):
            return tuple(freeze(x) for x in v)
        return v
    return tuple(sorted((k, freeze(v)) for k, v in flags.items()))


def _pack_bias_cols(params):
    """[128, NB, 220] f32 per-partition bias columns (q biases pre-scaled)."""
    out = np.zeros((128, NB, _BC_W), np.float32)

    def put(b, base, vec, scale=1.0):
        v = np.asarray(vec, np.float32).reshape(-1) * scale
        ncols = v.size // 128
        out[:, b, base:base + ncols] = v.reshape(ncols, 128).T

    for b, bk in enumerate(params["blocks"]):
        put(b, _BC_Q_S, bk["a1"]["bq"], RSQ)   # [8,512] -> 32 cols
        put(b, _BC_K_S, bk["a1"]["bk"])
        put(b, _BC_V_S, bk["a1"]["bv"])
        put(b, _BC_O_S, bk["a1"]["bo"])
        put(b, _BC_Q_C, bk["a2"]["bq"], RSQ)
        put(b, _BC_K_C, bk["a2"]["bk"])
        put(b, _BC_V_C, bk["a2"]["bv"])
        put(b, _BC_O_C, bk["a2"]["bo"])
        put(b, _BC_B1, bk["bias1"])
        put(b, _BC_B2, bk["bias2"])
    return out


def _pack_lngb(params):
    out = np.zeros((128, NB, 3, 2, DK), np.float32)
    for b, bk in enumerate(params["blocks"]):
        for i, (g, bb) in enumerate((("g1", "b1"), ("g2", "b2"), ("g3", "b3"))):
            out[:, b, i, 0, :] = np.asarray(bk[g], np.float32).reshape(DK, 128).T
            out[:, b, i, 1, :] = np.asarray(bk[bb], np.float32).reshape(DK, 128).T
    return out


_NC_CACHE = {}


def _get_nc(flags):
    key = _flags_key(flags)
    if key not in _NC_CACHE:
        _NC_CACHE[key] = build_decoder_nc(flags)
    return _NC_CACHE[key]


def _to_bf(a):
    return np.ascontiguousarray(np.asarray(a, np.float32)).astype(BF_NP)


def prepare_in_maps(encoded_source, source_padding, target, params):
    encoded_source = np.asarray(encoded_source, np.float32)
    source_padding = np.asarray(source_padding, np.float32)
    target = np.asarray(target)
    emb = np.asarray(params["emb"], np.float32)

    pe = _pos_enc(T, D)
    tp = (target != V).astype(np.float32)                       # [B,T]
    tril = np.tril(np.ones((T, T), np.float32))

    mask2_full = tp[:, :, None] * source_padding[:, None, :]     # [B,T,S]
    use_mask2 = bool(np.any(mask2_full == 0))
    flags = _compute_flags(params, use_mask2)
    flags["mask1_pure_causal"] = bool(np.all(tp == 1.0))

    shared = {
        "bias_cols": _pack_bias_cols(params),
        "lngb": _pack_lngb(params),
        "wout": _to_bf(params["Wout"]),
    }
    for b, bk in enumerate(params["blocks"]):
        for tag, att in (("s", "a1"), ("c", "a2")):
            w = bk[att]
            shared[f"wqkv_{tag}{b}"] = np.stack([
                _to_bf(w["Wq"]), _to_bf(w["Wk"]), _to_bf(w["Wv"])])  # [3,H,D,D]
            shared[f"wo_{tag}{b}"] = _to_bf(w["Wo"])
        shared[f"w1_{b}"] = _to_bf(bk["W1"])
        shared[f"w2_{b}"] = _to_bf(bk["W2"])

    in_maps = []
    for bidx in range(B):
        x0 = emb[target[bidx]] + pe                              # [T,D] f32
        m1 = tril * np.outer(tp[bidx], tp[bidx])
        im = dict(shared)
        im["x0t"] = np.ascontiguousarray(x0.T.astype(np.float32))
        im["encst"] = np.ascontiguousarray(encoded_source[bidx].T).astype(BF_NP)
        im["mask1"] = np.where(m1 == 0, np.float32(NEG), np.float32(0.0))
        if use_mask2:
            im["mask2"] = np.where(mask2_full[bidx] == 0, np.float32(NEG), np.float32(0.0))
        in_maps.append(im)
    return in_maps, flags


def gather_outputs(results, params):
    y = np.stack([r["y"] for r in results], 0).astype(np.float32)  # [B,T,VOC]
    bout = np.asarray(params["bout"], np.float32)
    if np.any(bout != 0):
        y = y + bout[None, None, :]
    att = np.stack([r["attw"].reshape(H, T, S) for r in results], 0)
    return y, att


class _Runner:
    """Cached jitted SPMD executor over jax.devices()[:8] with repeat-timing
    support (outputs recycled as donated buffers)."""

    def __init__(self, nc):
        import jax
        from concourse import bass2jax as B2J
        from jax.experimental.shard_map import shard_map
        from jax.sharding import Mesh, PartitionSpec, NamedSharding

        B2J.install_neuronx_cc_hook()
        self.nc = nc
        partition_name = nc.partition_id_tensor.name if nc.partition_id_tensor else None
        in_names, out_names, out_avals, zero_outs = [], [], [], []
        for alloc in nc.m.functions[0].allocations:
            if not isinstance(alloc, mybir.MemoryLocationSet):
                continue
            name = alloc.memorylocations[0].name
            if alloc.kind == "ExternalInput":
                if name != partition_name:
                    in_names.append(name)
            elif alloc.kind == "ExternalOutput":
                out_names.append(name)
                shape = tuple(alloc.tensor_shape)
                dtype = mybir.dt.np(alloc.dtype)
                out_avals.append(jax.core.ShapedArray(shape, dtype))
                zero_outs.append(np.zeros(shape, dtype))
        self.in_names = list(in_names)
        self.out_names = out_names
        self.zero_outs = zero_outs
        n_params = len(in_names)
        n_outs = len(out_avals)
        all_in = in_names + out_names + ([partition_name] if partition_name else [])

        def _body(*args):
            operands = list(args)
            if partition_name is not None:
                operands.append(B2J.partition_id_tensor())
            outs = B2J._bass_exec_p.bind(
                *operands,
                out_avals=tuple(out_avals),
                in_names=tuple(all_in),
                out_names=tuple(out_names),
                lowering_input_output_aliases=(),
                sim_require_finite=True,
                sim_require_nnan=True,
                nc=nc,
            )
            return tuple(outs)

        devices = jax.devices()[:B]
        assert len(devices) == B
        self.mesh = Mesh(np.asarray(devices), ("core",))
        self.spec = PartitionSpec("core")
        self.sharding = NamedSharding(self.mesh, self.spec)
        in_specs = (self.spec,) * (n_params + n_outs)
        out_specs = (self.spec,) * n_outs
        donate = tuple(range(n_params, n_params + n_outs))
        self.fn = jax.jit(
            shard_map(_body, mesh=self.mesh, in_specs=in_specs,
                      out_specs=out_specs, check_rep=False),
            donate_argnums=donate, keep_unused=True,
        )
        self._dev_in = None
        self._jax = jax

    def put_inputs(self, in_maps):
        jax = self._jax
        concat = [np.concatenate([np.asarray(m[n]) for m in in_maps], axis=0)
                  for n in self.in_names]
        self._dev_in = [jax.device_put(a, self.sharding) for a in concat]

    def _zeros_dev(self):
        jax = self._jax
        import jax.numpy as jnp
        if not hasattr(self, "_zfn"):
            shapes = [((B * z.shape[0],) + z.shape[1:], z.dtype) for z in self.zero_outs]
            self._zfn = jax.jit(
                lambda: tuple(jnp.zeros(s, d) for s, d in shapes),
                out_shardings=tuple(self.sharding for _ in shapes))
        return list(self._zfn())

    def run(self):
        outs = self.fn(*self._dev_in, *self._zeros_dev())
        self._jax.block_until_ready(outs)
        host = [np.asarray(o) for o in outs]
        results = []
        for c in range(B):
            r = {}
            for i, name in enumerate(self.out_names):
                r[name] = host[i].reshape(B, host[i].shape[0] // B, *host[i].shape[1:])[c]
            results.append(r)
        return results

    def time(self, reps=8):
        import time as _t
        outs = self.fn(*self._dev_in, *self._zeros_dev())
        self._jax.block_until_ready(outs)
        best = float("inf")
        for _ in range(reps):
            t0 = _t.perf_counter()
            outs = self.fn(*self._dev_in, *outs)
            self._jax.block_until_ready(outs)
            best = min(best, _t.perf_counter() - t0)
        return best * 1e9

    def _run_k(self, k):
        import time as _t
        o = self._zeros_dev()
        t0 = _t.perf_counter()
        for _ in range(k):
            o = self.fn(*self._dev_in, *o)
        self._jax.block_until_ready(o)
        return _t.perf_counter() - t0

    def time_slope(self, k1=4, k2=24, reps=3):
        """per-execution time from the marginal cost of extra chained runs;
        removes the axon round-trip latency (~80ms) from the estimate."""
        outs = self.fn(*self._dev_in, *self._zeros_dev())
        self._jax.block_until_ready(outs)
        del outs
        best = float("inf")
        for _ in range(reps):
            t1 = self._run_k(k1)
            t2 = self._run_k(k2)
            best = min(best, (t2 - t1) / (k2 - k1))
        return best * 1e9


_RUNNER_CACHE = {}


def _get_runner(flags):
    key = _flags_key(flags)
    if key not in _RUNNER_CACHE:
        _RUNNER_CACHE[key] = _Runner(_get_nc(flags))
    return _RUNNER_CACHE[key]


def kernel(encoded_source, source_padding, target, params):
    in_maps, flags = prepare_in_maps(encoded_source, source_padding, target, params)
    runner = _get_runner(flags)
    runner.put_inputs(in_maps)
    return gather_outputs(runner.run(), params)


def time_kernel(encoded_source, source_padding, target, params, reps=8):
    in_maps, flags = prepare_in_maps(encoded_source, source_padding, target, params)
    runner = _get_runner(flags)
    runner.put_inputs(in_maps)
    return runner.time_slope()


# revision 39
# speedup vs baseline: 1.6283x; 1.1705x over previous
"""Trainium2 Bass kernel for a 4-block transformer decoder (nn_Decoder).

Strategy: data-parallel over batch across 8 NeuronCores (1 batch element per
core), no collectives. Per core the whole decoder runs on [T=256, D=512]
activations kept feature-major ("TN layout": features on SBUF partitions,
tokens on the free dim), so every linear layer consumes weights as the
stationary matmul operand directly in their natural [in, out] layout.
Weights are converted to bf16 on the host (halves HBM traffic; fp32 matmul
on TRN2 runs at 1/4 rate); accumulation stays fp32 in PSUM and the residual
stream / softmax / layernorm statistics stay fp32.
"""

import numpy as np
import ml_dtypes

import bass_rust
import concourse.bass as bass
import concourse.mybir as mybir
from concourse.tile import TileContext
from concourse.masks import make_identity
from concourse.bass_utils import run_bass_kernel_spmd

DT = mybir.dt
BF = DT.bfloat16
F32 = DT.float32
AF = mybir.ActivationFunctionType
AX = mybir.AxisListType
OP = mybir.AluOpType
BF_NP = ml_dtypes.bfloat16

# Model dims (fixed by the problem)
V = 32000
D = 512
H = 8
NB = 4
B = 8
S = 256
T = 256
DK = D // 128          # 4 k-tiles over the model dim
TK = T // 128          # 2 token tiles
HD = H * D             # 4096 concat-head dim
FF = 4 * D             # 2048
VOC = V + 1            # 32001
VCH = 512              # vocab free-dim chunk
NEG = -1.0e30
RSQ = 1.0 / float(np.sqrt(np.float32(D)))

# bias-column layout inside bias_cols[:, block, col]
_BC_Q_S, _BC_K_S, _BC_V_S, _BC_O_S = 0, 32, 64, 96
_BC_Q_C, _BC_K_C, _BC_V_C, _BC_O_C = 100, 132, 164, 196
_BC_B1, _BC_B2 = 200, 216
_BC_W = 220


def _split_excess_waits(nc, max_waits=1):
    """walrus in this container encodes at most one semaphore wait per
    instruction; move extra waits onto same-engine carrier nops."""
    for bb in nc.main_func.blocks:
        insts = bb.instructions
        def nwaits(ins):
            si = ins.sync_info
            return len(si.on_wait) if si is not None else 0
        if not any(nwaits(i) > max_waits for i in insts):
            continue
        new_list = []
        for ins in list(insts):
            si0 = ins.sync_info
            waits = list(si0.on_wait) if si0 is not None else []
            if len(waits) > max_waits:
                excess = waits[: len(waits) - max_waits]
                keep = waits[len(waits) - max_waits:]
                eng = nc.engines[ins.engine]
                for i in range(0, len(excess), max_waits):
                    chunk = excess[i:i + max_waits]
                    carrier = eng.nop(nofuse=True, hint="wait_split")
                    cins = carrier.ins
                    cur = nc.cur_bb.bb.instructions
                    assert cur[-1].name == cins.name
                    cur.pop()
                    cins.sync_info = bass_rust.SyncInfo(on_wait=chunk, on_update=[])
                    new_list.append(cins)
                si0.on_wait = keep
            new_list.append(ins)
        insts[:] = new_list


def build_decoder_nc(flags):
    """flags: dict with booleans use_mask2, bias_* (see _compute_flags)."""
    nc = bass.Bass()

    x0t = nc.declare_dram_parameter("x0t", [D, T], F32, isOutput=False)
    encst = nc.declare_dram_parameter("encst", [D, S], BF, isOutput=False)
    mask1 = nc.declare_dram_parameter("mask1", [T, T], F32, isOutput=False)
    if flags["use_mask2"]:
        mask2 = nc.declare_dram_parameter("mask2", [T, S], F32, isOutput=False)
    bias_cols = nc.declare_dram_parameter("bias_cols", [128, NB, _BC_W], F32, isOutput=False)
    lngb = nc.declare_dram_parameter("lngb", [128, NB, 3, 2, DK], F32, isOutput=False)
    wqkv_s = [nc.declare_dram_parameter(f"wqkv_s{b}", [3, H, D, D], BF, isOutput=False) for b in range(NB)]
    wqkv_c = [nc.declare_dram_parameter(f"wqkv_c{b}", [3, H, D, D], BF, isOutput=False) for b in range(NB)]
    wo_s = [nc.declare_dram_parameter(f"wo_s{b}", [HD, D], BF, isOutput=False) for b in range(NB)]
    wo_c = [nc.declare_dram_parameter(f"wo_c{b}", [HD, D], BF, isOutput=False) for b in range(NB)]
    w1 = [nc.declare_dram_parameter(f"w1_{b}", [D, FF], BF, isOutput=False) for b in range(NB)]
    w2 = [nc.declare_dram_parameter(f"w2_{b}", [FF, D], BF, isOutput=False) for b in range(NB)]
    wout = nc.declare_dram_parameter("wout", [D, VOC], BF, isOutput=False)

    y = nc.declare_dram_parameter("y", [T, VOC], DT.float16, isOutput=True)
    attw = nc.declare_dram_parameter("attw", [H * T, S], F32, isOutput=True)

    with TileContext(nc) as tc:
        with (
            tc.tile_pool(name="const", bufs=1) as const,
            tc.tile_pool(name="persist", bufs=1) as persist,
            tc.tile_pool(name="wpool", bufs=2) as wpool,
            tc.tile_pool(name="apool", bufs=2) as apool,
            tc.tile_pool(name="sm", bufs=3) as sm,
            tc.tile_pool(name="pmid", bufs=3, space="PSUM") as pmid,
            tc.tile_pool(name="pbig", bufs=2, space="PSUM") as pbig,
            tc.tile_pool(name="ptr", bufs=2, space="PSUM") as ptr,
            tc.tile_pool(name="pst", bufs=1, space="PSUM") as pst,
        ):
            # ---- constants ----
            ident = const.tile([128, 128], BF)
            make_identity(nc, ident)
            ones_col = const.tile([128, 1], BF)
            nc.vector.memset(ones_col, 1.0)
            ones_row_f = const.tile([1, 128], F32)
            nc.vector.memset(ones_row_f, 1.0)
            ones_row_b = const.tile([1, 128], BF)
            nc.vector.memset(ones_row_b, 1.0)
            eps1 = const.tile([1, 1], F32)
            nc.vector.memset(eps1, 1e-5)

            # ---- persistent state ----
            res = persist.tile([128, DK, T], F32)       # residual stream (xT)
            xnbf = persist.tile([128, DK, T], BF)       # current matmul input (xT, bf16)
            encst_sb = persist.tile([128, DK, S], BF)
            obf = persist.tile([128, H * DK, T], BF)    # concat head outputs (oT)
            h1t = persist.tile([128, FF // 128, T], BF)
            tbf = persist.tile([128, DK, T], BF)        # LN scratch: bf16 copy of res
            sqbf = persist.tile([128, DK, T], BF)       # LN scratch: squares
            mask1_sb = persist.tile([128, TK, T], F32)
            if flags["use_mask2"]:
                mask2_sb = persist.tile([128, TK, S], F32)
            bias_sb = persist.tile([128, NB, _BC_W], F32)
            lngb_sb = persist.tile([128, NB, 3, 2, DK], F32)

            # ---- input DMA ----
            nc.sync.dma_start(out=res, in_=x0t.rearrange("(kt p) t -> p kt t", p=128))
            nc.sync.dma_start(out=encst_sb, in_=encst.rearrange("(kt p) t -> p kt t", p=128))
            nc.sync.dma_start(out=mask1_sb, in_=mask1.rearrange("(qt p) k -> p qt k", p=128))
            if flags["use_mask2"]:
                nc.sync.dma_start(out=mask2_sb, in_=mask2.rearrange("(qt p) k -> p qt k", p=128))
            nc.sync.dma_start(out=bias_sb, in_=bias_cols[:, :, :])
            nc.sync.dma_start(out=lngb_sb, in_=lngb[:, :, :, :, :])

            for c in range(DK):
                nc.gpsimd.tensor_copy(xnbf[:, c, :], res[:, c, :])

            def bias_col(b, base, idx):
                return bias_sb[:, b, base + idx:base + idx + 1]

            def ln_col(b, i, gb, c):
                return lngb_sb[:, b, i, gb, c:c + 1]

            # ---------- building blocks ----------

            def proj_tn(wtile, x_in, out_tile, out_idx, b, bias_base, h,
                        scale, use_bias, engine_alt=False):
                """TN projection: out[:, out_idx+n, :] = W.T @ x (+bias) for n in 0..DK-1."""
                for n in range(DK):
                    ps = pmid.tile([128, T], F32, tag="pmid", name="ps_proj")
                    for kt in range(DK):
                        nc.tensor.matmul(ps, wtile[:, kt, n * 128:(n + 1) * 128],
                                         x_in[:, kt, :], start=(kt == 0), stop=(kt == DK - 1))
                    dst = out_tile[:, out_idx + n, :]
                    if use_bias:
                        bcol = bias_col(b, bias_base, (h * DK + n) if h is not None else n)
                        nc.scalar.activation(out=dst, in_=ps, func=AF.Identity,
                                             bias=bcol, scale=scale)
                    elif scale != 1.0:
                        nc.scalar.mul(dst, ps, scale)
                    elif engine_alt:
                        nc.vector.tensor_copy(dst, ps)
                    else:
                        nc.scalar.copy(dst, ps)

            def v_proj(wtile, x_in, vout):
                """natural-layout value projection: v[t, e] (bias folded in at oT)."""
                for tk in range(TK):
                    ps = pbig.tile([128, D], F32, tag="pbig", name="ps_v")
                    for kt in range(DK):
                        nc.tensor.matmul(ps, x_in[:, kt, tk * 128:(tk + 1) * 128],
                                         wtile[:, kt, :], start=(kt == 0), stop=(kt == DK - 1))
                    nc.vector.tensor_copy(vout[:, tk, :], ps)

            def attention(b, is_self, emit_att, pstat, pre=None):
                """one MHA sublayer; reads xnbf (queries) and for cross the
                encoder states; accumulates output into res via Wo.

                Head loop is software-pipelined: head h+1's K/V/Q projections
                are emitted between head h's softmax and its transpose/oT so
                the PE stays busy during the softmax latency. K/V are
                projected before Q so that for cross-attention the (LN-
                independent) K/V matmuls can run while the preceding
                layernorm finishes."""
                wqkv = wqkv_s[b] if is_self else wqkv_c[b]
                wo = wo_s[b] if is_self else wo_c[b]
                kvx = xnbf if is_self else encst_sb
                klen = T if is_self else S
                kk = klen // 128
                q_base = _BC_Q_S if is_self else _BC_Q_C
                k_base = _BC_K_S if is_self else _BC_K_C
                v_base = _BC_V_S if is_self else _BC_V_C
                o_base = _BC_O_S if is_self else _BC_O_C
                ub_q = flags["bias_q_s" if is_self else "bias_q_c"][b]
                ub_k = flags["bias_k_s" if is_self else "bias_k_c"][b]
                ub_v = flags["bias_v_s" if is_self else "bias_v_c"][b]
                ub_o = flags["bias_o_s" if is_self else "bias_o_c"][b]
                msk = None
                if is_self:
                    msk = mask1_sb
                elif flags["use_mask2"]:
                    msk = mask2_sb

                def load_head(h):
                    # split K/V (used first) from Q so their prefetch depths
                    # decouple
                    wkv = wpool.tile([128, 2, DK, D], BF, tag="wkv", bufs=4, name="wkv")
                    for j, m in enumerate((1, 2)):
                        eng = nc.sync if (h + j) % 2 == 0 else nc.scalar
                        eng.dma_start(out=wkv[:, m - 1], in_=wqkv[m, h].rearrange(
                            "(kt p) n -> p kt n", p=128))
                    wq = wpool.tile([128, DK, D], BF, tag="wq", bufs=3, name="wq")
                    eng = nc.sync if h % 2 == 0 else nc.scalar
                    eng.dma_start(out=wq, in_=wqkv[0, h].rearrange(
                        "(kt p) n -> p kt n", p=128))
                    return wkv, wq

                def proj_kv(h, wkv):
                    kbf = apool.tile([128, DK, S], BF, tag="kbf", bufs=2, name="kbf")
                    vbf = apool.tile([128, kk, D], BF, tag="vbf", bufs=3, name="vbf")
                    proj_tn(wkv[:, 0], kvx, kbf, 0, b, k_base, h, 1.0, ub_k, engine_alt=True)
                    v_proj(wkv[:, 1], kvx, vbf)
                    return kbf, vbf

                def proj_head(h, wt, kv=None):
                    wkv, wq = wt
                    kbf, vbf = kv if kv is not None else proj_kv(h, wkv)
                    qbf = apool.tile([128, DK, T], BF, tag="qbf", bufs=2, name="qbf")
                    proj_tn(wq, xnbf, qbf, 0, b, q_base, h, RSQ, ub_q)
                    return qbf, kbf, vbf

                if pre is not None:
                    wt, kv0 = pre
                    nxt = proj_head(0, wt, kv0)
                else:
                    wt = load_head(0)
                    nxt = proj_head(0, wt)
                wt_next = load_head(1)
                for h in range(H):
                    qbf, kbf, vbf = nxt
                    # pure causal self-attention: queries in tile 0 only attend
                    # keys in tile 0, so skip the upper-right score block
                    causal = is_self and flags["mask1_pure_causal"]
                    wbf = apool.tile([128, TK, klen], BF, tag="wbf", bufs=3, name="wbf")
                    for qt in range(TK):
                        kw = 128 if (causal and qt == 0) else klen
                        ps = pmid.tile([128, klen], F32, tag="pmid", name="ps_sc")
                        for et in range(DK):
                            nc.tensor.matmul(ps[:, :kw], qbf[:, et, qt * 128:(qt + 1) * 128],
                                             kbf[:, et, :kw], start=(et == 0), stop=(et == DK - 1))
                        if msk is not None:
                            if causal and qt == 1:
                                nc.vector.tensor_add(ps[:, 128:256], ps[:, 128:256],
                                                     msk[:, 1, 128:256])
                            else:
                                nc.vector.tensor_add(ps[:, :kw], ps[:, :kw], msk[:, qt, :kw])
                        nmx = sm.tile([128, 1], F32, tag="nmx", bufs=4, name="nmx")
                        nc.vector.tensor_reduce(out=nmx, in_=ps[:, :kw], axis=AX.X,
                                                op=OP.max, negate=True)
                        den = sm.tile([128, 1], F32, tag="den", bufs=4, name="den")
                        if emit_att:
                            p32 = sm.tile([128, klen], F32, tag="p32", bufs=2, name="p32")
                            nc.scalar.activation(out=p32, in_=ps, func=AF.Exp,
                                                 bias=nmx, accum_out=den)
                            rden = sm.tile([128, 1], F32, tag="rden", bufs=4, name="rden")
                            nc.vector.reciprocal(rden, den)
                            nc.vector.tensor_scalar_mul(out=wbf[:, qt, :], in0=p32, scalar1=rden)
                            att32 = sm.tile([128, klen], F32, tag="att32", bufs=2, name="att32")
                            nc.vector.tensor_scalar_mul(out=att32, in0=p32, scalar1=rden)
                            nc.gpsimd.dma_start(out=attw[h * T + qt * 128:h * T + (qt + 1) * 128, :],
                                                in_=att32)
                        else:
                            pbf = sm.tile([128, klen], BF, tag="pbf", bufs=2, name="pbf")
                            nc.scalar.activation(out=pbf[:, :kw], in_=ps[:, :kw], func=AF.Exp,
                                                 bias=nmx, accum_out=den)
                            rden = sm.tile([128, 1], F32, tag="rden", bufs=4, name="rden")
                            nc.vector.reciprocal(rden, den)
                            nc.vector.tensor_scalar_mul(out=wbf[:, qt, :kw], in0=pbf[:, :kw],
                                                        scalar1=rden)

                    # pipeline: next head's projections run while this head's
                    # softmax results drain through transpose/oT; weights are
                    # prefetched two heads ahead
                    if h + 1 < H:
                        nxt = proj_head(h + 1, wt_next)
                        if h + 2 < H:
                            wt_next = load_head(h + 2)

                    # transpose attention weights: wT[k, q]
                    wtbf = apool.tile([128, kk, T], BF, tag="wtbf", bufs=3, name="wtbf")
                    for qt in range(TK):
                        for k2 in range(kk):
                            if causal and qt == 0 and k2 == 1:
                                continue        # zero block — never read below
                            pt = ptr.tile([128, 128], BF, tag="ptr", name="pt")
                            nc.tensor.transpose(pt, wbf[:, qt, k2 * 128:(k2 + 1) * 128], ident)
                            nc.vector.tensor_copy(wtbf[:, k2, qt * 128:(qt + 1) * 128], pt)

                    # oT[e, q] = v.T @ wT  (+bv via softmax rows summing to 1)
                    for e in range(DK):
                        ps = pmid.tile([128, T], F32, tag="pmid", name="ps_o")
                        esl = slice(e * 128, (e + 1) * 128)
                        if causal:
                            # q-tile 0 sees only k-tile 0; q-tile 1 sees both
                            nc.tensor.matmul(ps[:, 0:128], vbf[:, 0, esl],
                                             wtbf[:, 0, 0:128], start=True, stop=True,
                                             skip_group_check=True)
                            nc.tensor.matmul(ps[:, 128:256], vbf[:, 0, esl],
                                             wtbf[:, 0, 128:256], start=True, stop=False,
                                             skip_group_check=True)
                            nc.tensor.matmul(ps[:, 128:256], vbf[:, 1, esl],
                                             wtbf[:, 1, 128:256], start=False, stop=True,
                                             skip_group_check=True)
                        else:
                            for k2 in range(kk):
                                nc.tensor.matmul(ps, vbf[:, k2, esl],
                                                 wtbf[:, k2, :], start=(k2 == 0),
                                                 stop=(k2 == kk - 1))
                        dst = obf[:, h * DK + e, :]
                        if ub_v:
                            nc.scalar.activation(out=dst, in_=ps, func=AF.Identity,
                                                 bias=bias_col(b, v_base, h * DK + e), scale=1.0)
                        else:
                            nc.scalar.copy(dst, ps)

                # output projection + residual add (+ incremental LN stats)
                wog = []
                for g in range(4):
                    wg = wpool.tile([128, 8, D], BF, tag="wo", bufs=4, name="wo_g")
                    eng = nc.sync if g % 2 == 0 else nc.scalar
                    eng.dma_start(out=wg, in_=wo[g * 1024:(g + 1) * 1024, :]
                                  .rearrange("(j p) n -> p j n", p=128))
                    wog.append(wg)
                for n in range(DK):
                    ps = pmid.tile([128, T], F32, tag="pmid", name="ps_wo")
                    for het in range(HD // 128):
                        nc.tensor.matmul(ps, wog[het // 8][:, het % 8, n * 128:(n + 1) * 128],
                                         obf[:, het, :], start=(het == 0), stop=(het == HD // 128 - 1))
                    if ub_o:
                        asb = sm.tile([128, T], F32, tag="asb", bufs=1, name="asb")
                        nc.scalar.activation(out=asb, in_=ps, func=AF.Identity,
                                             bias=bias_col(b, o_base, n), scale=1.0)
                        nc.vector.tensor_add(tbf[:, n, :], res[:, n, :], asb)
                        nc.vector.tensor_add(res[:, n, :], res[:, n, :], asb)
                    else:
                        nc.vector.tensor_add(tbf[:, n, :], res[:, n, :], ps)
                        nc.vector.tensor_add(res[:, n, :], res[:, n, :], ps)
                    res_stats_chunk(n, pstat)

            def res_stats_chunk(c, pstat):
                """tbf[:, c, :] (bf16 copy of the new residual) was written by
                the dual-add above; fold its column-sum / sum-of-squares into
                the LN stats psum."""
                nc.gpsimd.tensor_mul(sqbf[:, c, :], tbf[:, c, :], tbf[:, c, :])
                nc.tensor.matmul(pstat[0:1, 0, :], ones_col, tbf[:, c, :],
                                 start=(c == 0), stop=(c == DK - 1), skip_group_check=True)
                nc.tensor.matmul(pstat[0:1, 1, :], ones_col, sqbf[:, c, :],
                                 start=False, stop=(c == DK - 1), skip_group_check=True)

            def layernorm(b, i, pstat):
                """finish LN from accumulated stats: res -> LN(res), xnbf=bf16.
                The mean is broadcast immediately so the (res - mean) subtracts
                overlap the rstd chain (msq/var on DVE, ln/exp on ACT)."""
                me = sm.tile([1, 2, T], F32, tag="srow", bufs=1, name="me")
                nc.vector.tensor_scalar_mul(out=me, in0=pstat, scalar1=1.0 / D)
                mean = me[0:1, 0, :]
                e2 = me[0:1, 1, :]
                bc = pbig.tile([128, 2, T], F32, tag="pbig", name="bc")
                nc.tensor.matmul(bc[:, 1, :], ones_row_f, mean, start=True, stop=True)
                tmps = []
                for c in range(DK):
                    tmp = sm.tile([128, T], F32, tag="lntmp", bufs=4, name="lntmp")
                    nc.vector.tensor_sub(tmp, res[:, c, :], bc[:, 1, :])
                    tmps.append(tmp)
                # rstd chain computed in place over e2: var -> ln -> rstd
                msq = sm.tile([1, T], F32, tag="msq", bufs=1, name="msq")
                nc.vector.tensor_mul(msq, mean, mean)
                nc.vector.tensor_sub(e2, e2, msq)
                nc.scalar.activation(out=e2, in_=e2, func=AF.Ln, bias=eps1)
                rstd = e2
                nc.scalar.activation(out=rstd, in_=rstd, func=AF.Exp, scale=-0.5)
                nc.tensor.matmul(bc[:, 0, :], ones_row_f, rstd, start=False, stop=True,
                                 skip_group_check=True)
                # post-norm: the residual stream itself becomes the LN output
                affine = flags["ln_affine"][b][i]
                for c in range(DK):
                    # write the bf16 matmul input directly (shortest path to the
                    # next projection); refresh the f32 residual lazily from it
                    nc.vector.tensor_mul(xnbf[:, c, :], tmps[c], bc[:, 0, :])
                    if affine:
                        nc.vector.tensor_scalar(out=xnbf[:, c, :], in0=xnbf[:, c, :],
                                                scalar1=ln_col(b, i, 0, c),
                                                scalar2=ln_col(b, i, 1, c),
                                                op0=OP.mult, op1=OP.add)
                    nc.gpsimd.tensor_copy(res[:, c, :], xnbf[:, c, :])

            def ffn_prefetch(b):
                w1k = []
                for kt in range(DK):
                    wt = wpool.tile([128, FF], BF, tag="w1", bufs=5, name="w1_t")
                    eng = nc.sync if kt % 2 == 0 else nc.scalar
                    eng.dma_start(out=wt, in_=w1[b][kt * 128:(kt + 1) * 128, :])
                    w1k.append(wt)
                return w1k

            def ffn(b, pstat, w1k):
                ub1 = flags["bias_b1"][b]
                for n in range(FF // 128):
                    ps = pmid.tile([128, T], F32, tag="pmid", name="ps_h1")
                    for kt in range(DK):
                        nc.tensor.matmul(ps, w1k[kt][:, n * 128:(n + 1) * 128],
                                         xnbf[:, kt, :], start=(kt == 0), stop=(kt == DK - 1))
                    if ub1:
                        nc.scalar.activation(out=h1t[:, n, :], in_=ps, func=AF.Relu,
                                             bias=bias_col(b, _BC_B1, n), scale=1.0)
                    else:
                        nc.scalar.activation(out=h1t[:, n, :], in_=ps, func=AF.Relu)
                w2k = []
                for i in range(8):
                    wt = wpool.tile([128, 2, D], BF, tag="w2", bufs=8, name="w2_t")
                    nc.sync.dma_start(out=wt, in_=w2[b][i * 256:(i + 1) * 256, :]
                                      .rearrange("(j p) n -> p j n", p=128))
                    w2k.append(wt)
                ub2 = flags["bias_b2"][b]
                for n in range(DK):
                    ps = pmid.tile([128, T], F32, tag="pmid", name="ps_h2")
                    for mt in range(FF // 128):
                        nc.tensor.matmul(ps, w2k[mt // 2][:, mt % 2, n * 128:(n + 1) * 128],
                                         h1t[:, mt, :], start=(mt == 0), stop=(mt == FF // 128 - 1))
                    if ub2:
                        asb = sm.tile([128, T], F32, tag="asb", bufs=1, name="asb2")
                        nc.scalar.activation(out=asb, in_=ps, func=AF.Identity,
                                             bias=bias_col(b, _BC_B2, n), scale=1.0)
                        nc.vector.tensor_add(tbf[:, n, :], res[:, n, :], asb)
                        nc.vector.tensor_add(res[:, n, :], res[:, n, :], asb)
                    else:
                        nc.vector.tensor_add(tbf[:, n, :], res[:, n, :], ps)
                        nc.vector.tensor_add(res[:, n, :], res[:, n, :], ps)
                    res_stats_chunk(n, pstat)

            # ---------- the decoder ----------
            def new_pstat():
                return pst.tile([1, 2, T], F32, tag="pst", name="pstat")

            def cross_pre(b):
                # head-0 K/V of cross-attention only needs the encoder states,
                # so emit it before the preceding LN finishes to keep PE busy
                wkv = wpool.tile([128, 2, DK, D], BF, tag="wkv", bufs=4, name="wkv_pre")
                for j, m in enumerate((1, 2)):
                    eng = nc.sync if j % 2 == 0 else nc.scalar
                    eng.dma_start(out=wkv[:, m - 1], in_=wqkv_c[b][m, 0].rearrange(
                        "(kt p) n -> p kt n", p=128))
                wq = wpool.tile([128, DK, D], BF, tag="wq", bufs=3, name="wq_pre")
                nc.scalar.dma_start(out=wq, in_=wqkv_c[b][0, 0].rearrange(
                    "(kt p) n -> p kt n", p=128))
                kbf = apool.tile([128, DK, S], BF, tag="kbf", bufs=2, name="kbf_pre")
                vbf = apool.tile([128, TK, D], BF, tag="vbf", bufs=3, name="vbf_pre")
                ub_k = flags["bias_k_c"][b]
                proj_tn(wkv[:, 0], encst_sb, kbf, 0, b, _BC_K_C, 0, 1.0, ub_k, engine_alt=True)
                v_proj(wkv[:, 1], encst_sb, vbf)
                return (wkv, wq), (kbf, vbf)

            for b in range(NB):
                pstat = new_pstat()
                attention(b, is_self=True, emit_att=False, pstat=pstat)
                pre = cross_pre(b)
                layernorm(b, 0, pstat)
                pstat = new_pstat()
                attention(b, is_self=False, emit_att=(b == 0), pstat=pstat, pre=pre)
                w1k = ffn_prefetch(b)
                layernorm(b, 1, pstat)
                pstat = new_pstat()
                ffn(b, pstat, w1k)
                layernorm(b, 2, pstat)

            # ---------- vocab projection (natural layout) ----------
            # 256-wide psum groups alternating across both psum pools give a
            # 5-slot rotation that decouples MMs / copies / output DMAs
            nch = (VOC + VCH - 1) // VCH
            galt = [0]
            for ci in range(nch):
                c0 = ci * VCH
                cw = min(VCH, VOC - c0)
                wt = wpool.tile([128, DK, VCH], BF, tag="wout", bufs=3, name="wout_t")
                nc.sync.dma_start(out=wt[:, :, :cw],
                                  in_=wout[:, c0:c0 + cw].rearrange("(kt p) n -> p kt n", p=128))
                for qt in range(TK):
                    ysb = sm.tile([128, VCH], DT.float16, tag="ysb", bufs=4, name="ysb")
                    for half in range(2):
                        h0 = half * 256
                        hw = min(256, cw - h0)
                        if hw <= 0:
                            continue
                        galt[0] ^= 1
                        if galt[0]:
                            ps = pmid.tile([128, T], F32, tag="pmid", name="ps_y")
                        else:
                            ps = pbig.tile([128, VCH], F32, tag="pbig", name="ps_y2")
                        for kt in range(DK):
                            nc.tensor.matmul(ps[:, :hw], xnbf[:, kt, qt * 128:(qt + 1) * 128],
                                             wt[:, kt, h0:h0 + hw], start=(kt == 0),
                                             stop=(kt == DK - 1))
                        if half == 0:
                            nc.scalar.copy(ysb[:, h0:h0 + hw], ps[:, :hw])
                        else:
                            nc.vector.tensor_copy(ysb[:, h0:h0 + hw], ps[:, :hw])
                    nc.scalar.dma_start(out=y[qt * 128:(qt + 1) * 128, c0:c0 + cw],
                                        in_=ysb[:, :cw])

    _split_excess_waits(nc)
    return nc


# ---------------------------------------------------------------------------
# host side
# ---------------------------------------------------------------------------

def _pos_enc(seq_len, dim):
    pos = np.arange(seq_len, dtype=np.float32)[:, None]
    den = np.exp(np.arange(0, dim, 2, dtype=np.float32) * (-np.log(10000.0) / dim))
    ang = (pos * den).astype(np.float32)
    pe = np.zeros((seq_len, dim), np.float32)
    pe[:, 0::2] = np.sin(ang)
    pe[:, 1::2] = np.cos(ang)
    return pe


def _nz(a):
    return bool(np.any(np.asarray(a) != 0))


def _compute_flags(params, use_mask2):
    blocks = params["blocks"]
    flags = {
        "use_mask2": use_mask2,
        "bias_q_s": [_nz(bk["a1"]["bq"]) for bk in blocks],
        "bias_k_s": [_nz(bk["a1"]["bk"]) for bk in blocks],
        "bias_v_s": [_nz(bk["a1"]["bv"]) for bk in blocks],
        "bias_o_s": [_nz(bk["a1"]["bo"]) for bk in blocks],
        "bias_q_c": [_nz(bk["a2"]["bq"]) for bk in blocks],
        "bias_k_c": [_nz(bk["a2"]["bk"]) for bk in blocks],
        "bias_v_c": [_nz(bk["a2"]["bv"]) for bk in blocks],
        "bias_o_c": [_nz(bk["a2"]["bo"]) for bk in blocks],
        "bias_b1": [_nz(bk["bias1"]) for bk in blocks],
        "bias_b2": [_nz(bk["bias2"]) for bk in blocks],
        "ln_affine": [[
            _nz(np.asarray(bk[g]) - 1.0) or _nz(bk[bb])
            for g, bb in (("g1", "b1"), ("g2", "b2"), ("g3", "b3"))
        ] for bk in blocks],
    }
    return flags


def _flags_key(flags):
    def freeze(v):
        if isinstance(v, list):
            return tuple(freeze(x) for x in v)
        return v
    return tuple(sorted((k, freeze(v)) for k, v in flags.items()))


def _pack_bias_cols(params):
    """[128, NB, 220] f32 per-partition bias columns (q biases pre-scaled)."""
    out = np.zeros((128, NB, _BC_W), np.float32)

    def put(b, base, vec, scale=1.0):
        v = np.asarray(vec, np.float32).reshape(-1) * scale
        ncols = v.size // 128
        out[:, b, base:base + ncols] = v.reshape(ncols, 128).T

    for b, bk in enumerate(params["blocks"]):
        put(b, _BC_Q_S, bk["a1"]["bq"], RSQ)   # [8,512] -> 32 cols
        put(b, _BC_K_S, bk["a1"]["bk"])
        put(b, _BC_V_S, bk["a1"]["bv"])
        put(b, _BC_O_S, bk["a1"]["bo"])
        put(b, _BC_Q_C, bk["a2"]["bq"], RSQ)
        put(b, _BC_K_C, bk["a2"]["bk"])
        put(b, _BC_V_C, bk["a2"]["bv"])
        put(b, _BC_O_C, bk["a2"]["bo"])
        put(b, _BC_B1, bk["bias1"])
        put(b, _BC_B2, bk["bias2"])
    return out


def _pack_lngb(params):
    out = np.zeros((128, NB, 3, 2, DK), np.float32)
    for b, bk in enumerate(params["blocks"]):
        for i, (g, bb) in enumerate((("g1", "b1"), ("g2", "b2"), ("g3", "b3"))):
            out[:, b, i, 0, :] = np.asarray(bk[g], np.float32).reshape(DK, 128).T
            out[:, b, i, 1, :] = np.asarray(bk[bb], np.float32).reshape(DK, 128).T
    return out


_NC_CACHE = {}


def _get_nc(flags):
    key = _flags_key(flags)
    if key not in _NC_CACHE:
        _NC_CACHE[key] = build_decoder_nc(flags)
    return _NC_CACHE[key]


def _to_bf(a):
    return np.ascontiguousarray(np.asarray(a, np.float32)).astype(BF_NP)


def prepare_in_maps(encoded_source, source_padding, target, params):
    encoded_source = np.asarray(encoded_source, np.float32)
    source_padding = np.asarray(source_padding, np.float32)
    target = np.asarray(target)
    emb = np.asarray(params["emb"], np.float32)

    pe = _pos_enc(T, D)
    tp = (target != V).astype(np.float32)                       # [B,T]
    tril = np.tril(np.ones((T, T), np.float32))

    mask2_full = tp[:, :, None] * source_padding[:, None, :]     # [B,T,S]
    use_mask2 = bool(np.any(mask2_full == 0))
    flags = _compute_flags(params, use_mask2)
    flags["mask1_pure_causal"] = bool(np.all(tp == 1.0))

    shared = {
        "bias_cols": _pack_bias_cols(params),
        "lngb": _pack_lngb(params),
        "wout": _to_bf(params["Wout"]),
    }
    for b, bk in enumerate(params["blocks"]):
        for tag, att in (("s", "a1"), ("c", "a2")):
            w = bk[att]
            shared[f"wqkv_{tag}{b}"] = np.stack([
                _to_bf(w["Wq"]), _to_bf(w["Wk"]), _to_bf(w["Wv"])])  # [3,H,D,D]
            shared[f"wo_{tag}{b}"] = _to_bf(w["Wo"])
        shared[f"w1_{b}"] = _to_bf(bk["W1"])
        shared[f"w2_{b}"] = _to_bf(bk["W2"])

    in_maps = []
    for bidx in range(B):
        x0 = emb[target[bidx]] + pe                              # [T,D] f32
        m1 = tril * np.outer(tp[bidx], tp[bidx])
        im = dict(shared)
        im["x0t"] = np.ascontiguousarray(x0.T.astype(np.float32))
        im["encst"] = np.ascontiguousarray(encoded_source[bidx].T).astype(BF_NP)
        im["mask1"] = np.where(m1 == 0, np.float32(NEG), np.float32(0.0))
        if use_mask2:
            im["mask2"] = np.where(mask2_full[bidx] == 0, np.float32(NEG), np.float32(0.0))
        in_maps.append(im)
    return in_maps, flags


def gather_outputs(results, params):
    y = np.stack([r["y"] for r in results], 0).astype(np.float32)  # [B,T,VOC]
    bout = np.asarray(params["bout"], np.float32)
    if np.any(bout != 0):
        y = y + bout[None, None, :]
    att = np.stack([r["attw"].reshape(H, T, S) for r in results], 0)
    return y, att


class _Runner:
    """Cached jitted SPMD executor over jax.devices()[:8] with repeat-timing
    support (outputs recycled as donated buffers)."""

    def __init__(self, nc):
        import jax
        from concourse import bass2jax as B2J
        from jax.experimental.shard_map import shard_map
        from jax.sharding import Mesh, PartitionSpec, NamedSharding

        B2J.install_neuronx_cc_hook()
        self.nc = nc
        partition_name = nc.partition_id_tensor.name if nc.partition_id_tensor else None
        in_names, out_names, out_avals, zero_outs = [], [], [], []
        for alloc in nc.m.functions[0].allocations:
            if not isinstance(alloc, mybir.MemoryLocationSet):
                continue
            name = alloc.memorylocations[0].name
            if alloc.kind == "ExternalInput":
                if name != partition_name:
                    in_names.append(name)
            elif alloc.kind == "ExternalOutput":
                out_names.append(name)
                shape = tuple(alloc.tensor_shape)
                dtype = mybir.dt.np(alloc.dtype)
                out_avals.append(jax.core.ShapedArray(shape, dtype))
                zero_outs.append(np.zeros(shape, dtype))
        self.in_names = list(in_names)
        self.out_names = out_names
        self.zero_outs = zero_outs
        n_params = len(in_names)
        n_outs = len(out_avals)
        all_in = in_names + out_names + ([partition_name] if partition_name else [])

        def _body(*args):
            operands = list(args)
            if partition_name is not None:
                operands.append(B2J.partition_id_tensor())
            outs = B2J._bass_exec_p.bind(
                *operands,
                out_avals=tuple(out_avals),
                in_names=tuple(all_in),
                out_names=tuple(out_names),
                lowering_input_output_aliases=(),
                sim_require_finite=True,
                sim_require_nnan=True,
                nc=nc,
            )
            return tuple(outs)

        devices = jax.devices()[:B]
        assert len(devices) == B
        self.mesh = Mesh(np.asarray(devices), ("core",))
        self.spec = PartitionSpec("core")
        self.sharding = NamedSharding(self.mesh, self.spec)
        in_specs = (self.spec,) * (n_params + n_outs)
        out_specs = (self.spec,) * n_outs
        donate = tuple(range(n_params, n_params + n_outs))
        self.fn = jax.jit(
            shard_map(_body, mesh=self.mesh, in_specs=in_specs,
                      out_specs=out_specs, check_rep=False),
            donate_argnums=donate, keep_unused=True,
        )
        self._dev_in = None
        self._jax = jax

    def put_inputs(self, in_maps):
        jax = self._jax
        concat = [np.concatenate([np.asarray(m[n]) for m in in_maps], axis=0)
                  for n in self.in_names]
        self._dev_in = [jax.device_put(a, self.sharding) for a in concat]

    def _zeros_dev(self):
        jax = self._jax
        import jax.numpy as jnp
        if not hasattr(self, "_zfn"):
            shapes = [((B * z.shape[0],) + z.shape[1:], z.dtype) for z in self.zero_outs]
            self._zfn = jax.jit(
                lambda: tuple(jnp.zeros(s, d) for s, d in shapes),
                out_shardings=tuple(self.sharding for _ in shapes))
        return list(self._zfn())

    def run(self):
        outs = self.fn(*self._dev_in, *self._zeros_dev())
        self._jax.block_until_ready(outs)
        host = [np.asarray(o) for o in outs]
        results = []
        for c in range(B):
            r = {}
            for i, name in enumerate(self.out_names):
                r[name] = host[i].reshape(B, host[i].shape[0] // B, *host[i].shape[1:])[c]
            results.append(r)
        return results

    def time(self, reps=8):
        import time as _t
        outs = self.fn(*self._dev_in, *self._zeros_dev())
        self._jax.block_until_ready(outs)
        best = float("inf")
        for _ in range(reps):
            t0 = _t.perf_counter()
            outs = self.fn(*self._dev_in, *outs)
            self._jax.block_until_ready(outs)
            best = min(best, _t.perf_counter() - t0)
        return best * 1e9

    def _run_k(self, k):
        import time as _t
        o = self._zeros_dev()
        t0 = _t.perf_counter()
        for _ in range(k):
            o = self.fn(*self._dev_in, *o)
        self._jax.block_until_ready(o)
        return _t.perf_counter() - t0

    def time_slope(self, k1=4, k2=24, reps=3):
        """per-execution time from the marginal cost of extra chained runs;
        removes the axon round-trip latency (~80ms) from the estimate."""
        outs = self.fn(*self._dev_in, *self._zeros_dev())
        self._jax.block_until_ready(outs)
        del outs
        best = float("inf")
        for _ in range(reps):
            t1 = self._run_k(k1)
            t2 = self._run_k(k2)
            best = min(best, (t2 - t1) / (k2 - k1))
        return best * 1e9


_RUNNER_CACHE = {}


def _get_runner(flags):
    key = _flags_key(flags)
    if key not in _RUNNER_CACHE:
        _RUNNER_CACHE[key] = _Runner(_get_nc(flags))
    return _RUNNER_CACHE[key]


def kernel(encoded_source, source_padding, target, params):
    in_maps, flags = prepare_in_maps(encoded_source, source_padding, target, params)
    runner = _get_runner(flags)
    runner.put_inputs(in_maps)
    return gather_outputs(runner.run(), params)


def time_kernel(encoded_source, source_padding, target, params, reps=8):
    in_maps, flags = prepare_in_maps(encoded_source, source_padding, target, params)
    runner = _get_runner(flags)
    runner.put_inputs(in_maps)
    return runner.time_slope()
